# revision 1
# baseline (speedup 1.0000x reference)
"""Trainium2 Bass kernel for nn_CustomTransformer_50062138802561.

4-layer encoder (d=512, 8 heads, ffn 64) + fc to 128 + 64-step sequential
decoder (single shared layer, d=128, 8 heads dh=16, ffn 16).

Strategy:
- Data-parallel over batch: 8 cores x 4 batches each. No collectives.
- Decoder loop rewritten as incremental KV-cache decode (mathematically
  identical to the reference's full-recompute loop: padded zero rows produce
  k=b_k / v=b_v which we pre-fill in the cache).
- All weights pre-transposed on the host so every DMA load is contiguous.
- Encoder: token-major activations, feature-major operands via PE transposes.
- Decoder: key-major score layout (St = K^T Qblk via block-diag q), so no
  per-step transposes; softmax normalization deferred to a mask-expanded
  reciprocal applied after the AV matmul.
"""

import os
import numpy as np

import concourse.bass as bass
import concourse.mybir as mybir
from concourse import bacc
from concourse.tile import TileContext

F32 = mybir.dt.float32
F32R = mybir.dt.float32r
def _r(ap):
    return ap.bitcast(F32R)
AF = mybir.ActivationFunctionType
ALU = mybir.AluOpType

B, T, DIN, DOUT = 32, 64, 512, 128
NHEAD = 8
FF_ENC, FF_DEC, NLAYERS = 64, 16, 4
EPS = 1e-5
NCORES = 8
BL = B // NCORES          # local batch = 4
NTOK = BL * T             # 256 local encoder tokens
DHE = DIN // NHEAD        # 64 encoder head dim
DHD = DOUT // NHEAD       # 16 decoder head dim
NSTEP = int(os.environ.get("KERNEL_NSTEP", T))
NENC = int(os.environ.get("KERNEL_NENC", NLAYERS))
DEC_PHASE = int(os.environ.get("KERNEL_DEC_PHASE", 99))
ATT_PHASE = int(os.environ.get("KERNEL_ATT_PHASE", 99))

_CACHE = {}


def _patch_act_table_pass(nc):
    """All activation funcs we use (Exp, Ln, Square, Relu, Identity, Copy) live
    in the combined natural_log_exp_and_others table, but the auto-inserted
    loads alternate between the exp-only and ln-only sets (~1.3us each).
    Make every other set look empty so the insertion pass maps all
    activations to the combined set and hoists to a single load."""
    import types
    import bass_rust as _br
    from concourse.hw_specs import get_activation_tables

    def patched(self):
        has_activation = any(
            isinstance(i, mybir.InstActivation)
            for b in self.main_func.blocks
            for i in b.instructions
        )
        if not has_activation:
            return
        tabs = get_activation_tables(self.m.arch)
        keep = "natural_log_exp_and_others"
        for f in self.m.functions:
            for blk in f.blocks:
                for ins in blk.instructions:
                    if isinstance(ins, mybir.InstActivation):
                        assert ins.func in tabs[keep], f"{ins.func} not in {keep}"
        tables = [(k, (v if k == keep else set())) for k, v in tabs.items()]
        _br.insert_act_table_loads(self, tables)

    nc.insert_act_table_loads = types.MethodType(patched, nc)


def _split_drain_waits(nc, maxw=1):
    """Walrus in this container rejects >1 sync-wait on CTRL-class (Drain)
    instructions; split extras onto preceding nops on the same engine."""
    n = 0
    for f in nc.m.functions:
        for blk in f.blocks:
            newlist = []
            for ins in blk.instructions:
                si = ins.sync_info
                if si is not None and len(si.on_wait) > maxw and type(ins).__name__ == "InstDrain":
                    waits = list(si.on_wait)
                    for w in waits[:-maxw]:
                        nop = mybir.InstNoOp(name=f"Wsplit{n}", ins=[], outs=[])
                        n += 1
                        nop.engine = ins.engine
                        nop.sync_info = mybir.SyncInfo(on_wait=[w], on_update=[])
                        newlist.append(nop)
                    ins.sync_info = mybir.SyncInfo(on_wait=waits[-maxw:], on_update=list(si.on_update))
                newlist.append(ins)
            blk.instructions = newlist


def build_program():
    nc = bacc.Bacc("TRN2", target_bir_lowering=False, debug=False)
    D = {}

    def din(name, shape):
        D[name] = nc.dram_tensor(name, list(shape), F32, kind="ExternalInput").ap()
        return D[name]

    din("src_tok", [NTOK, DIN])
    din("srcT", [DIN, NTOK])
    for l in range(NLAYERS):
        din(f"e{l}_qkvT", [DIN, 3 * DIN])
        din(f"e{l}_bqk", [128, 8])
        din(f"e{l}_bv", [1, DIN])
        din(f"e{l}_woT", [DIN, DIN])
        din(f"e{l}_bo", [1, DIN])
        din(f"e{l}_f1T", [DIN, FF_ENC])
        din(f"e{l}_bf1", [FF_ENC, 1])
        din(f"e{l}_f2T", [FF_ENC, DIN])
        din(f"e{l}_bf2", [1, DIN])
        din(f"e{l}_g1", [1, DIN])
        din(f"e{l}_b1", [1, DIN])
        din(f"e{l}_g2", [1, DIN])
        din(f"e{l}_b2", [1, DIN])
    din("fcT", [DIN, DOUT])
    din("bfc", [1, DOUT])
    din("d_sqT", [DOUT, DOUT]); din("d_bsq", [DOUT, 1])
    din("d_skT", [DOUT, DOUT]); din("d_bsk", [DOUT, 1])
    din("d_svT", [DOUT, DOUT]); din("d_bsv", [1, DOUT]); din("d_bsvc", [DOUT, 1])
    din("d_soT", [DOUT, DOUT]); din("d_bso", [1, DOUT])
    din("d_cqT", [DOUT, DOUT]); din("d_bcq", [DOUT, 1])
    din("d_ckT", [DOUT, DOUT]); din("d_bck", [DOUT, 1])
    din("d_cvT", [DOUT, DOUT]); din("d_bcv", [1, DOUT])
    din("d_coT", [DOUT, DOUT]); din("d_bco", [1, DOUT])
    din("d_f1T", [DOUT, FF_DEC]); din("d_bf1", [FF_DEC, 1])
    din("d_f2T", [FF_DEC, DOUT]); din("d_bf2", [1, DOUT])
    for nm in ("g1", "b1", "g2", "b2", "g3", "b3"):
        din(f"d_{nm}", [1, DOUT])
    din("identity", [128, 128])
    din("mask", [128, NHEAD])
    din("maskT", [NHEAD, 128])
    din("ones", [128, 1])

    out_d = nc.dram_tensor("out", [BL, T, DOUT], F32, kind="ExternalOutput").ap()

    with TileContext(nc) as tc:
        _build_body(nc, tc, D, out_d)

    _patch_act_table_pass(nc)
    nc.compile()
    _split_drain_waits(nc)
    return nc, list(D.keys())


def _ln_tokmajor(nc, pool, pre, nparts, dfeat, g_b, b_b, out_ap, eps_ap, eng2=None,
                 dve_sq=False):
    """LayerNorm over the free dim of token-major `pre` [nparts, dfeat]."""
    ve = nc.vector
    e2 = eng2 or ve
    s1 = pool.tile([nparts, 1], F32, tag="ln_s1")
    ve.tensor_reduce(out=s1[:], in_=pre, op=ALU.add, axis=mybir.AxisListType.X)
    mu = pool.tile([nparts, 1], F32, tag="ln_mu")
    ve.tensor_scalar_mul(mu[:], s1[:], 1.0 / dfeat)
    sqj = pool.tile([nparts, dfeat], F32, tag="ln_sqj")
    s2 = pool.tile([nparts, 1], F32, tag="ln_s2")
    nc.scalar.activation(sqj[:], pre, AF.Square, accum_out=s2[:])
    mu2 = pool.tile([nparts, 1], F32, tag="ln_mu2")
    ve.tensor_mul(mu2[:], mu[:], mu[:])
    var = pool.tile([nparts, 1], F32, tag="ln_var")
    ve.tensor_scalar(var[:], s2[:], 1.0 / dfeat, mu2[:], op0=ALU.mult, op1=ALU.subtract)
    # rstd = exp(-0.5*ln(var+eps)): keeps ACT in the natural_log_exp func set
    lnv = pool.tile([nparts, 1], F32, tag="ln_lnv")
    nc.scalar.activation(lnv[:], var[:], AF.Ln, bias=eps_ap)
    al = pool.tile([nparts, 1], F32, tag="ln_al")
    nc.scalar.activation(al[:], lnv[:], AF.Exp, scale=-0.5)
    mup = pool.tile([nparts, 1], F32, tag="ln_mup")
    ve.tensor_scalar(mup[:], mu[:], al[:], -1.0, op0=ALU.mult, op1=ALU.mult)
    xn = pool.tile([nparts, dfeat], F32, tag="ln_xn")
    ve.tensor_scalar(xn[:], pre, al[:], mup[:], op0=ALU.mult, op1=ALU.add)
    xg = pool.tile([nparts, dfeat], F32, tag="ln_xg")
    ve.tensor_mul(xg[:], xn[:], g_b)
    e2.tensor_add(out_ap, xg[:], b_b)
    return out_ap


def _build_body(nc, tc, D, out_d):
    import contextlib
    ctx = contextlib.ExitStack()
    ectx = contextlib.ExitStack()
    with ctx:
        cpool = ctx.enter_context(tc.tile_pool(name="const", bufs=1))
        w2pool = ectx.enter_context(tc.tile_pool(name="wts2", bufs=2))
        w1pool = ectx.enter_context(tc.tile_pool(name="wts1", bufs=1))
        apool = ectx.enter_context(tc.tile_pool(name="acts", bufs=1))
        spool = ectx.enter_context(tc.tile_pool(name="small", bufs=3))

        ident = cpool.tile([128, 128], F32, tag="ident")
        nc.sync.dma_start(out=ident[:], in_=D["identity"])
        mask = cpool.tile([128, NHEAD], F32, tag="mask")
        nc.sync.dma_start(out=mask[:], in_=D["mask"])
        maskT = cpool.tile([NHEAD, 128], F32, tag="maskT")
        nc.sync.dma_start(out=maskT[:], in_=D["maskT"])
        ones = cpool.tile([128, 1], F32, tag="ones_t")
        nc.sync.dma_start(out=ones[:], in_=D["ones"])
        eps_t = cpool.tile([128, 1], F32, tag="eps_t")
        nc.vector.memset(eps_t[:], EPS)

        # ---------------- encoder ----------------
        X_tok, XT = [], []
        for tt in range(2):
            xt_ = apool.tile([128, DIN], F32, tag=f"X_tok{tt}")
            nc.sync.dma_start(out=xt_[:], in_=D["src_tok"][tt * 128:(tt + 1) * 128, :])
            X_tok.append(xt_[:])
        for c in range(4):
            xc = apool.tile([128, NTOK], F32, tag=f"XT{c}")
            nc.sync.dma_start(out=xc[:], in_=D["srcT"][c * 128:(c + 1) * 128, :])
            XT.append(xc[:])

        for l in range(NENC):
            X_tok, XT = _enc_layer(nc, tc, D, l, X_tok, XT,
                                   w2pool, w1pool, apool, spool, ident, eps_t)

        # ---------------- fc + memory K/V ----------------
        fcTs = []
        for c in range(4):
            t_ = w1pool.tile([128, DOUT], F32, tag=f"fcT{c}")
            nc.sync.dma_start(out=t_[:], in_=D["fcT"][c * 128:(c + 1) * 128, :])
            fcTs.append(t_)
        bfc_b = cpool.tile([128, DOUT], F32, tag="bfc_b")
        _bcast_row(nc, cpool, D["bfc"], bfc_b, 128, "bfc")

        ckT = cpool.tile([DOUT, DOUT], F32, tag="d_ckT")
        nc.sync.dma_start(out=ckT[:], in_=D["d_ckT"])
        bck = cpool.tile([DOUT, 1], F32, tag="d_bck")
        nc.sync.dma_start(out=bck[:], in_=D["d_bck"])
        cvT = cpool.tile([DOUT, DOUT], F32, tag="d_cvT")
        nc.sync.dma_start(out=cvT[:], in_=D["d_cvT"])
        bcv_b = cpool.tile([128, DOUT], F32, tag="bcv_b")
        _bcast_row(nc, cpool, D["d_bcv"], bcv_b, 128, "bcv")

        Kmem = cpool.tile([128, NTOK], F32, tag="Kmem")
        Vmem = [cpool.tile([T, DOUT], F32, tag=f"Vmem{b}", name=f"Vmem{b}") for b in range(BL)]
        with tc.tile_pool(name="psfc", bufs=2, space="PSUM") as psfc:
            mem_tok = []
            for tt in range(2):
                mp = psfc.tile([128, DOUT], F32, tag="mem")
                for c in range(4):
                    nc.tensor.matmul(mp[:], XT[c][:, tt * 128:(tt + 1) * 128], fcTs[c][:],
                                     start=(c == 0), stop=(c == 3))
                ms = apool.tile([128, DOUT], F32, tag=f"mem_tok{tt}")
                nc.vector.tensor_add(ms[:], mp[:], bfc_b[:])
                mem_tok.append(ms)
            memT = apool.tile([128, NTOK], F32, tag="memT")
            for tt in range(2):
                tp = psfc.tile([128, 128], F32, tag="memTp")
                nc.tensor.transpose(tp[:], mem_tok[tt][:], ident[:])
                nc.scalar.copy(memT[:, tt * 128:(tt + 1) * 128], tp[:])
            kmp = psfc.tile([128, NTOK], F32, tag="kmem")
            nc.tensor.matmul(kmp[:], ckT[:], memT[:], start=True, stop=True)
            nc.scalar.activation(Kmem[:], kmp[:], AF.Identity, bias=bck[:])
            for b in range(BL):
                vmp = psfc.tile([T, DOUT], F32, tag="vmem")
                nc.tensor.matmul(vmp[:], memT[:, b * T:(b + 1) * T], cvT[:],
                                 start=True, stop=True)
                nc.vector.tensor_add(Vmem[b][:], vmp[:], bcv_b[0:T, :])

        # ---------------- decoder prep ----------------
        dw = {}
        for nm in ("d_sqT", "d_skT", "d_svT", "d_soT", "d_cqT", "d_coT"):
            t_ = cpool.tile([DOUT, DOUT], F32, tag=nm)
            nc.sync.dma_start(out=t_[:], in_=D[nm])
            dw[nm] = t_
        d_f1T = cpool.tile([DOUT, FF_DEC], F32, tag="d_f1T")
        nc.sync.dma_start(out=d_f1T[:], in_=D["d_f1T"])
        d_f2T = cpool.tile([FF_DEC, DOUT], F32, tag="d_f2T")
        nc.sync.dma_start(out=d_f2T[:], in_=D["d_f2T"])
        bsq = cpool.tile([DOUT, 1], F32, tag="d_bsq")
        nc.sync.dma_start(out=bsq[:], in_=D["d_bsq"])
        bsk = cpool.tile([DOUT, 1], F32, tag="d_bsk")
        nc.sync.dma_start(out=bsk[:], in_=D["d_bsk"])
        bcq = cpool.tile([DOUT, 1], F32, tag="d_bcq")
        nc.sync.dma_start(out=bcq[:], in_=D["d_bcq"])
        d_bf1 = cpool.tile([FF_DEC, 1], F32, tag="d_bf1")
        nc.sync.dma_start(out=d_bf1[:], in_=D["d_bf1"])
        bvec = {}
        for nm in ("d_g1", "d_b1", "d_g2", "d_b2", "d_g3", "d_b3"):
            b_ = cpool.tile([BL, DOUT], F32, tag=f"bv_{nm}")
            _bcast_row(nc, cpool, D[nm], b_, BL, nm)
            bvec[nm] = b_
        rows = {}
        for nm in ("d_bso", "d_bco", "d_bf2"):
            r_ = cpool.tile([1, DOUT], F32, tag=f"row_{nm}")
            nc.sync.dma_start(out=r_[:], in_=D[nm])
            rows[nm] = r_
        ones_r = cpool.tile([1, 128], F32, tag="ones_r")
        nc.vector.memset(ones_r[:], 1.0)
        rows["ones_r"] = ones_r

        Kc = cpool.tile([128, BL * (T + 1)], F32, tag="Kc")
        nc.vector.tensor_copy(Kc[:], bsk[:].broadcast_to([128, BL * (T + 1)]))
        bsvc = cpool.tile([DOUT, 1], F32, tag="d_bsvc")
        nc.sync.dma_start(out=bsvc[:], in_=D["d_bsvc"])
        VcT = cpool.tile([128, BL * (T + 1)], F32, tag="VcT")
        nc.vector.tensor_copy(VcT[:], bsvc[:].broadcast_to([128, BL * (T + 1)]))

        ectx.close()   # release encoder-phase SBUF before the decode loop
        opool = ctx.enter_context(tc.tile_pool(name="outp", bufs=1))
        out_sb = opool.tile([BL, T * DOUT], F32, tag="out_sb")
        zero4 = cpool.tile([BL, DOUT], F32, tag="zero4")
        nc.vector.memset(zero4[:], 0.0)
        zeroT = cpool.tile([DOUT, BL], F32, tag="zeroT")
        nc.vector.memset(zeroT[:], 0.0)

        # ---------------- decode loop ----------------
        with tc.tile_pool(name="dstep", bufs=3) as dpool, \
             tc.tile_pool(name="psD", bufs=4, space="PSUM") as psD:
            x_tok = zero4[:]
            xT = zeroT[:]
            for t in range(NSTEP):
                x_tok, xT = _dec_step(nc, t, x_tok, xT, Kc, VcT, bsvc, Kmem, Vmem,
                                      dw, bsq, bsk, bcq, d_f1T, d_f2T, d_bf1, bvec,
                                      rows, mask, maskT, ones, ident, dpool, psD,
                                      out_sb, eps_t)

        nc.sync.dma_start(out=out_d.rearrange("b t d -> b (t d)"), in_=out_sb[:])


def _bcast_row(nc, cpool, dram_row, dst_tile, channels, key):
    row = cpool.tile([1, dram_row.shape[-1]], F32, tag=f"brow_{key}")
    nc.sync.dma_start(out=row[:], in_=dram_row)
    nc.gpsimd.partition_broadcast(dst_tile[:], row[:], channels=channels)


def _enc_layer(nc, tc, D, l, X_tok, XT, w2pool, w1pool, apool, spool, ident, eps_t):
    qkvT = []
    for c in range(4):
        t_ = w2pool.tile([128, 3 * DIN], F32, tag=f"qkvT{c}")
        nc.sync.dma_start(out=t_[:], in_=D[f"e{l}_qkvT"][c * 128:(c + 1) * 128, :])
        qkvT.append(t_)
    woT = []
    for c in range(4):
        t_ = w1pool.tile([128, DIN], F32, tag=f"woT{c}")
        nc.sync.dma_start(out=t_[:], in_=D[f"e{l}_woT"][c * 128:(c + 1) * 128, :])
        woT.append(t_)
    f1T = []
    for c in range(4):
        t_ = w1pool.tile([128, FF_ENC], F32, tag=f"f1T{c}")
        nc.sync.dma_start(out=t_[:], in_=D[f"e{l}_f1T"][c * 128:(c + 1) * 128, :])
        f1T.append(t_)
    f2T = w1pool.tile([FF_ENC, DIN], F32, tag="f2T")
    nc.sync.dma_start(out=f2T[:], in_=D[f"e{l}_f2T"])
    bqk = w1pool.tile([128, 8], F32, tag="bqk")
    nc.sync.dma_start(out=bqk[:], in_=D[f"e{l}_bqk"])
    bf1 = w1pool.tile([FF_ENC, 1], F32, tag="bf1")
    nc.sync.dma_start(out=bf1[:], in_=D[f"e{l}_bf1"])
    bb = {}
    for nm in ("bv", "bo", "bf2", "g1", "b1", "g2", "b2"):
        b_ = w1pool.tile([128, DIN], F32, tag=f"bb_{nm}")
        row = w1pool.tile([1, DIN], F32, tag=f"bbrow_{nm}")
        nc.sync.dma_start(out=row[:], in_=D[f"e{l}_{nm}"])
        nc.gpsimd.partition_broadcast(b_[:], row[:], channels=128)
        bb[nm] = b_

    # --- QKV ---
    QK = []
    V = []
    with tc.tile_pool(name=f"ps_qkv{l}", bufs=2, space="PSUM") as psq:
        for m in range(8):
            pq = psq.tile([128, NTOK], F32, tag="qk")
            for c in range(4):
                nc.tensor.matmul(pq[:], qkvT[c][:, m * 128:(m + 1) * 128], XT[c],
                                 start=(c == 0), stop=(c == 3))
            qs = apool.tile([128, NTOK], F32, tag=f"QK{m}")
            nc.scalar.activation(qs[:], pq[:], AF.Identity, bias=bqk[:, m:m + 1])
            QK.append(qs)
        for b in range(BL):
            pv = psq.tile([T, DIN], F32, tag="v")
            for c in range(4):
                nc.tensor.matmul(pv[:], XT[c][:, b * T:(b + 1) * T],
                                 qkvT[c][:, 2 * DIN:3 * DIN],
                                 start=(c == 0), stop=(c == 3))
            vs = apool.tile([T, DIN], F32, tag=f"V{b}", name=f"Vb{b}")
            nc.vector.tensor_add(vs[:], pv[:], bb["bv"][0:T, :])
            V.append(vs)

    # --- attention ---
    OT = [apool.tile([128, NTOK], F32, tag=f"OT{c}", name=f"OT{c}") for c in range(4)]
    with tc.tile_pool(name=f"ps_att{l}", bufs=2, space="PSUM") as psa, \
         tc.tile_pool(name=f"sb_att{l}", bufs=4) as sba:
        for b in range(BL):
            den = spool.tile([T, NHEAD], F32, tag="den")
            Ps = []
            for h in range(NHEAD):
                c, r0 = h // 2, (h % 2) * DHE
                Qs = QK[c][r0:r0 + DHE, b * T:(b + 1) * T]
                Ks = QK[4 + c][r0:r0 + DHE, b * T:(b + 1) * T]
                sp = psa.tile([T, T], F32, tag="S", bufs=3)
                nc.tensor.matmul(sp[:], Qs, Ks, start=True, stop=True)
                p_ = sba.tile([T, T], F32, tag="P", bufs=9)
                nc.scalar.activation(p_[:], sp[:], AF.Exp, scale=1.0 / np.sqrt(DHE),
                                     accum_out=den[:, h:h + 1])
                Ps.append(p_)
            rb = spool.tile([T, NHEAD], F32, tag="rb")
            nc.vector.reciprocal(rb[:], den[:])
            for h in range(NHEAD):
                c, r0 = h // 2, (h % 2) * DHE
                a_ = sba.tile([T, T], F32, tag="A")
                nc.scalar.activation(a_[:], Ps[h][:], AF.Copy, scale=rb[:, h:h + 1])
                atp = psa.tile([T, T], F32, tag="AT")
                nc.tensor.transpose(atp[:], a_[:], ident[0:T, 0:T])
                ats = sba.tile([T, T], F32, tag="ATs")
                nc.vector.tensor_copy(ats[:], atp[:])
                avp = psa.tile([DHE, T], F32, tag="AV")
                Vs = V[b][0:T, h * DHE:(h + 1) * DHE]
                nc.tensor.matmul(avp[:], Vs, ats[:], start=True, stop=True)
                nc.scalar.copy(OT[c][r0:r0 + DHE, b * T:(b + 1) * T], avp[:])

    # --- out-proj + residual + LN1 ---
    X1_tok = []
    X1T = [apool.tile([128, NTOK], F32, tag=f"X1T{c}", name=f"X1T{c}") for c in range(4)]
    with tc.tile_pool(name=f"ps_o{l}", bufs=2, space="PSUM") as pso:
        for tt in range(2):
            ap_ = pso.tile([128, DIN], F32, tag="ao")
            for c in range(4):
                nc.tensor.matmul(ap_[:], OT[c][:, tt * 128:(tt + 1) * 128], woT[c][:],
                                 start=(c == 0), stop=(c == 3))
            t1 = apool.tile([128, DIN], F32, tag=f"pre1_{tt}")
            nc.vector.tensor_add(t1[:], ap_[:], X_tok[tt])
            nc.vector.tensor_add(t1[:], t1[:], bb["bo"][:])
            x1 = apool.tile([128, DIN], F32, tag=f"X1_{tt}")
            _ln_tokmajor(nc, spool, t1[:], 128, DIN, bb["g1"][:], bb["b1"][:], x1[:], eps_t[:])
            X1_tok.append(x1[:])
        for tt in range(2):
            for c in range(4):
                tp = pso.tile([128, 128], F32, tag="xT")
                nc.tensor.transpose(tp[:], X1_tok[tt][:, c * 128:(c + 1) * 128], ident[:])
                nc.scalar.copy(X1T[c][:, tt * 128:(tt + 1) * 128], tp[:])

    # --- FFN + LN2 ---
    X2_tok = []
    X2T = [apool.tile([128, NTOK], F32, tag=f"X2T{c}", name=f"X2T{c}") for c in range(4)]
    with tc.tile_pool(name=f"ps_f{l}", bufs=2, space="PSUM") as psf:
        hp = psf.tile([FF_ENC, NTOK], F32, tag="h")
        for c in range(4):
            nc.tensor.matmul(hp[:], f1T[c][:], X1T[c][:], start=(c == 0), stop=(c == 3))
        hs = apool.tile([FF_ENC, NTOK], F32, tag="H")
        nc.scalar.activation(hs[:], hp[:], AF.Relu, bias=bf1[:])
        for tt in range(2):
            fp = psf.tile([128, DIN], F32, tag="f")
            nc.tensor.matmul(fp[:], hs[:, tt * 128:(tt + 1) * 128], f2T[:],
                             start=True, stop=True)
            t2 = apool.tile([128, DIN], F32, tag=f"pre2_{tt}")
            nc.vector.tensor_add(t2[:], fp[:], X1_tok[tt])
            nc.vector.tensor_add(t2[:], t2[:], bb["bf2"][:])
            x2 = apool.tile([128, DIN], F32, tag=f"X2_{tt}")
            _ln_tokmajor(nc, spool, t2[:], 128, DIN, bb["g2"][:], bb["b2"][:], x2[:], eps_t[:])
            X2_tok.append(x2[:])
        for tt in range(2):
            for c in range(4):
                tp = psf.tile([128, 128], F32, tag="xT2")
                nc.tensor.transpose(tp[:], X2_tok[tt][:, c * 128:(c + 1) * 128], ident[:])
                nc.scalar.copy(X2T[c][:, tt * 128:(tt + 1) * 128], tp[:])

    return X2_tok, X2T


def _attn_dec(nc, xT_ap, dpool, psD, qT, bq, K_ap, V_list, vlen,
              mask, maskT, ones, brow_o, oT, x_tok_ap, scale, pfx, ident4, ones14):
    """Decoder attention sublayer. Returns pre-LN residual tile AP [BL, DOUT]."""
    qp = psD.tile([DOUT, BL], F32, tag="pa")
    nc.tensor.matmul(qp[:], qT[:], xT_ap, start=True, stop=True)
    q_ = dpool.tile([DOUT, BL], F32, tag=f"{pfx}q")
    nc.vector.tensor_scalar_add(q_[:], qp[:], bq[:])
    def _bail():
        d = dpool.tile([BL, DOUT], F32, tag=f"{pfx}pre")
        nc.vector.tensor_copy(d[:], x_tok_ap)
        return d
    if ATT_PHASE < 2:
        return _bail()
    qblk = dpool.tile([128, BL * NHEAD], F32, tag=f"{pfx}qblk")
    nc.vector.tensor_mul(
        qblk[:].rearrange("p (b h) -> p b h", b=BL),
        q_[:].unsqueeze(2).broadcast_to([128, BL, NHEAD]),
        mask[:].unsqueeze(1).broadcast_to([128, BL, NHEAD]))
    if ATT_PHASE < 3:
        return _bail()
    stp = psD.tile([vlen, BL * NHEAD], F32, tag="pb")
    for b in range(BL):
        nc.tensor.matmul(stp[:, b * NHEAD:(b + 1) * NHEAD],
                         K_ap[:, b * vlen:(b + 1) * vlen],
                         qblk[:, b * NHEAD:(b + 1) * NHEAD], start=True, stop=True)
    if ATT_PHASE < 4:
        return _bail()
    pt = dpool.tile([vlen, BL * NHEAD], F32, tag=f"{pfx}pt")
    nc.scalar.activation(pt[:], stp[:], AF.Exp, scale=scale)
    if ATT_PHASE < 5:
        return _bail()
    denp = psD.tile([NHEAD, BL], F32, tag="pb")
    for b in range(BL):
        nc.tensor.matmul(denp[:, b:b + 1], pt[:, b * NHEAD:(b + 1) * NHEAD],
                         ones[0:vlen, :], start=True, stop=True)
    r_ = dpool.tile([NHEAD, BL], F32, tag=f"{pfx}r")
    nc.vector.reciprocal(r_[:], denp[:])
    if ATT_PHASE < 6:
        return _bail()
    erp = psD.tile([128, BL], F32, tag="pb")
    nc.tensor.matmul(erp[:], maskT[:], r_[:], start=True, stop=True)
    if ATT_PHASE < 7:
        return _bail()
    avp = psD.tile([128, BL * NHEAD], F32, tag="pb")
    for b in range(BL):
        nc.tensor.matmul(avp[:, b * NHEAD:(b + 1) * NHEAD], V_list[b],
                         pt[:, b * NHEAD:(b + 1) * NHEAD], start=True, stop=True)
    if ATT_PHASE < 8:
        return _bail()
    avm = dpool.tile([128, BL * NHEAD], F32, tag=f"{pfx}avm")
    nc.vector.tensor_mul(
        avm[:].rearrange("p (b h) -> p b h", b=BL),
        avp[:].rearrange("p (b h) -> p b h", b=BL),
        mask[:].unsqueeze(1).broadcast_to([128, BL, NHEAD]))
    o_ = dpool.tile([128, BL], F32, tag=f"{pfx}o")
    nc.vector.tensor_reduce(out=o_[:], in_=avm[:].rearrange("p (b h) -> p b h", b=BL),
                            op=ALU.add, axis=mybir.AxisListType.X)
    on = dpool.tile([128, BL], F32, tag=f"{pfx}on")
    nc.vector.tensor_mul(on[:], o_[:], erp[:])
    if ATT_PHASE < 9:
        return _bail()
    pp = psD.tile([BL, DOUT], F32, tag="pa")
    nc.tensor.matmul(pp[:], ident4, x_tok_ap, start=True, stop=False)
    nc.tensor.matmul(pp[:], ones14, brow_o, start=False, stop=False)
    nc.tensor.matmul(pp[:], on[:], oT[:], start=False, stop=True)
    pre = dpool.tile([BL, DOUT], F32, tag=f"{pfx}pre")
    nc.vector.tensor_copy(pre[:], pp[:])
    return pre


def _dec_step(nc, t, x_tok, xT, Kc, VcT, bsvc, Kmem, Vmem, dw, bsq, bsk, bcq,
              d_f1T, d_f2T, d_bf1, bvec, rows, mask, maskT, ones, ident,
              dpool, psD, out_sb, eps_t):
    # k projection straight into cache columns (strided over b)
    kp = psD.tile([DOUT, BL], F32, tag="pa")
    nc.tensor.matmul(kp[:], dw["d_skT"][:], xT, start=True, stop=True)
    kslice = Kc[:].rearrange("p (b j) -> p b j", b=BL)[:, :, t]
    nc.scalar.activation(kslice, kp[:], AF.Identity, bias=bsk[:])
    # v projection feature-major straight into VcT cache columns
    vp = psD.tile([DOUT, BL], F32, tag="pa")
    nc.tensor.matmul(vp[:], dw["d_svT"][:], xT, start=True, stop=True)
    vslice = VcT[:].rearrange("p (b j) -> p b j", b=BL)[:, :, t]
    nc.scalar.activation(vslice, vp[:], AF.Identity, bias=bsvc[:])
    if DEC_PHASE < 2:
        return x_tok, xT
    # transpose cache to key-major for the AV matmul (4 slices into one psum tile)
    ident4 = ident[0:BL, 0:BL]
    ones14 = rows["ones_r"][0:1, 0:BL]
    vtp = psD.tile([T + 1, BL * DOUT], F32, tag="pb")
    for b in range(BL):
        nc.tensor.transpose(vtp[:, b * DOUT:(b + 1) * DOUT],
                            VcT[:, b * (T + 1):(b + 1) * (T + 1)], ident[:])
    vcb = dpool.tile([T + 1, BL * DOUT], F32, tag="vcb")
    nc.vector.tensor_copy(vcb[:], vtp[:])
    Vcb = [vcb[:, b * DOUT:(b + 1) * DOUT] for b in range(BL)]

    if DEC_PHASE < 3:
        return x_tok, xT
    pre1 = _attn_dec(nc, xT, dpool, psD, dw["d_sqT"], bsq, Kc[:],
                     Vcb, T + 1, mask, maskT, ones,
                     rows["d_bso"][:], dw["d_soT"], x_tok, 1.0 / np.sqrt(DHD), "sa",
                     ident4, ones14)
    if DEC_PHASE < 4:
        return x_tok, xT
    x1 = dpool.tile([BL, DOUT], F32, tag="x1")
    _ln_tokmajor(nc, dpool, pre1[:], BL, DOUT, bvec["d_g1"][:], bvec["d_b1"][:],
                 x1[:], eps_t[0:BL, :], dve_sq=True)
    x1Tp = psD.tile([DOUT, BL], F32, tag="pa")
    nc.tensor.transpose(x1Tp[:], x1[:], ident4)
    x1T = dpool.tile([DOUT, BL], F32, tag="x1T")
    nc.vector.tensor_copy(x1T[:], x1Tp[:])

    if DEC_PHASE < 5:
        return x_tok, xT
    Vmem_list = [Vmem[b][:] for b in range(BL)]
    pre2 = _attn_dec(nc, x1T[:], dpool, psD, dw["d_cqT"], bcq, Kmem[:],
                     Vmem_list, T, mask, maskT, ones,
                     rows["d_bco"][:], dw["d_coT"], x1[:], 1.0 / np.sqrt(DHD), "ca",
                     ident4, ones14)
    x2 = dpool.tile([BL, DOUT], F32, tag="x2")
    _ln_tokmajor(nc, dpool, pre2[:], BL, DOUT, bvec["d_g2"][:], bvec["d_b2"][:],
                 x2[:], eps_t[0:BL, :], dve_sq=True)
    x2Tp = psD.tile([DOUT, BL], F32, tag="pa")
    nc.tensor.transpose(x2Tp[:], x2[:], ident4)
    x2T = dpool.tile([DOUT, BL], F32, tag="x2T")
    nc.vector.tensor_copy(x2T[:], x2Tp[:])

    if DEC_PHASE < 6:
        return x_tok, xT
    hp = psD.tile([FF_DEC, BL], F32, tag="pa")
    nc.tensor.matmul(hp[:], d_f1T[:], x2T[:], start=True, stop=True)
    h_ = dpool.tile([FF_DEC, BL], F32, tag="hdec")
    nc.scalar.activation(h_[:], hp[:], AF.Relu, bias=d_bf1[:])
    fp = psD.tile([BL, DOUT], F32, tag="pa")
    nc.tensor.matmul(fp[:], ident4, x2[:], start=True, stop=False)
    nc.tensor.matmul(fp[:], ones14, rows["d_bf2"][:], start=False, stop=False)
    nc.tensor.matmul(fp[:], h_[:], d_f2T[:], start=False, stop=True)
    pre3 = dpool.tile([BL, DOUT], F32, tag="pre3")
    nc.vector.tensor_copy(pre3[:], fp[:])
    xo_ap = out_sb[:, t * DOUT:(t + 1) * DOUT]
    _ln_tokmajor(nc, dpool, pre3[:], BL, DOUT, bvec["d_g3"][:], bvec["d_b3"][:],
                 xo_ap, eps_t[0:BL, :], dve_sq=True)
    xoTp = psD.tile([DOUT, BL], F32, tag="pa")
    nc.tensor.transpose(xoTp[:], xo_ap, ident4)
    xoT = dpool.tile([DOUT, BL], F32, tag="xoT")
    nc.vector.tensor_copy(xoT[:], xoTp[:])
    return xo_ap, xoT[:]


# ------------------------------------------------------------------
# host side
# ------------------------------------------------------------------

def _prep_shared(inputs):
    f = np.ascontiguousarray
    S = {}
    for l in range(NLAYERS):
        qkv_w = inputs["enc_qkv_w"][l]
        S[f"e{l}_qkvT"] = f(qkv_w.T)
        qkv_b = inputs["enc_qkv_b"][l]
        S[f"e{l}_bqk"] = f(qkv_b[:2 * DIN].reshape(8, 128).T)
        S[f"e{l}_bv"] = f(qkv_b[2 * DIN:].reshape(1, DIN))
        S[f"e{l}_woT"] = f(inputs["enc_out_w"][l].T)
        S[f"e{l}_bo"] = f(inputs["enc_out_b"][l].reshape(1, DIN))
        S[f"e{l}_f1T"] = f(inputs["enc_ff1_w"][l].T)
        S[f"e{l}_bf1"] = f(inputs["enc_ff1_b"][l].reshape(FF_ENC, 1))
        S[f"e{l}_f2T"] = f(inputs["enc_ff2_w"][l].T)
        S[f"e{l}_bf2"] = f(inputs["enc_ff2_b"][l].reshape(1, DIN))
        S[f"e{l}_g1"] = f(inputs["enc_ln1_g"][l].reshape(1, DIN))
        S[f"e{l}_b1"] = f(inputs["enc_ln1_b"][l].reshape(1, DIN))
        S[f"e{l}_g2"] = f(inputs["enc_ln2_g"][l].reshape(1, DIN))
        S[f"e{l}_b2"] = f(inputs["enc_ln2_b"][l].reshape(1, DIN))
    S["fcT"] = f(inputs["fc_w"].T)
    S["bfc"] = f(inputs["fc_b"].reshape(1, DOUT))
    sq, sk, sv = np.split(inputs["dec_sa_qkv_w"], 3, axis=0)
    bq_, bk_, bv_ = np.split(inputs["dec_sa_qkv_b"], 3)
    S["d_sqT"] = f(sq.T); S["d_bsq"] = f(bq_.reshape(DOUT, 1))
    S["d_skT"] = f(sk.T); S["d_bsk"] = f(bk_.reshape(DOUT, 1))
    S["d_svT"] = f(sv.T); S["d_bsv"] = f(bv_.reshape(1, DOUT)); S["d_bsvc"] = f(bv_.reshape(DOUT, 1))
    S["d_soT"] = f(inputs["dec_sa_out_w"].T)
    S["d_bso"] = f(inputs["dec_sa_out_b"].reshape(1, DOUT))
    cq, ck, cv = np.split(inputs["dec_ca_qkv_w"], 3, axis=0)
    cbq, cbk, cbv = np.split(inputs["dec_ca_qkv_b"], 3)
    S["d_cqT"] = f(cq.T); S["d_bcq"] = f(cbq.reshape(DOUT, 1))
    S["d_ckT"] = f(ck.T); S["d_bck"] = f(cbk.reshape(DOUT, 1))
    S["d_cvT"] = f(cv.T); S["d_bcv"] = f(cbv.reshape(1, DOUT))
    S["d_coT"] = f(inputs["dec_ca_out_w"].T)
    S["d_bco"] = f(inputs["dec_ca_out_b"].reshape(1, DOUT))
    S["d_f1T"] = f(inputs["dec_ff1_w"].T)
    S["d_bf1"] = f(inputs["dec_ff1_b"].reshape(FF_DEC, 1))
    S["d_f2T"] = f(inputs["dec_ff2_w"].T)
    S["d_bf2"] = f(inputs["dec_ff2_b"].reshape(1, DOUT))
    for nm in ("g1", "b1", "g2", "b2", "g3", "b3"):
        S[f"d_{nm}"] = f(inputs[f"dec_ln{nm[1]}_{nm[0]}"].reshape(1, DOUT))
    S["identity"] = np.eye(128, dtype=np.float32)
    S["mask"] = (np.arange(128)[:, None] // DHD == np.arange(NHEAD)[None, :]).astype(np.float32)
    S["maskT"] = f(S["mask"].T)
    S["ones"] = np.ones((128, 1), dtype=np.float32)
    return {k: np.asarray(v, dtype=np.float32) for k, v in S.items()}


def make_in_maps(inputs):
    shared = _prep_shared(inputs)
    src = np.asarray(inputs["src"], dtype=np.float32)
    in_maps = []
    for c in range(NCORES):
        shard = np.ascontiguousarray(src[c * BL:(c + 1) * BL])
        tok = shard.reshape(NTOK, DIN)
        m = dict(shared)
        m["src_tok"] = np.ascontiguousarray(tok)
        m["srcT"] = np.ascontiguousarray(tok.T)
        in_maps.append(m)
    return in_maps


def kernel(**inputs) -> np.ndarray:
    from concourse.bass_utils import run_bass_kernel_spmd
    if "nc" not in _CACHE:
        _CACHE["nc"] = build_program()[0]
    nc = _CACHE["nc"]
    in_maps = make_in_maps(inputs)
    res = run_bass_kernel_spmd(nc, in_maps, core_ids=list(range(NCORES)))
    out = np.concatenate([r["out"] for r in res.results], axis=0)
    return out.astype(np.float32)



# revision 13
# speedup vs baseline: 1.1835x; 1.1835x over previous
"""Trainium2 Bass kernel for nn_CustomTransformer_50062138802561.

4-layer encoder (d=512, 8 heads, ffn 64) + fc to 128 + 64-step sequential
decoder (single shared layer, d=128, 8 heads dh=16, ffn 16).

Strategy:
- Data-parallel over batch: 8 cores x 4 batches each. No collectives.
- Decoder loop rewritten as incremental KV-cache decode (mathematically
  identical to the reference's full-recompute loop: padded zero rows produce
  k=b_k / v=b_v which we pre-fill / fold into the out-proj bias).
- Decoder residual stream kept feature-major [128, BL]; residual adds,
  biases and projections all accumulate as PSUM matmuls, so no per-sublayer
  transposes.
- LayerNorm via the fused gpsimd partition-axis layernorm (one Pool op per
  batch column).
- V cache stores v-delta (v minus bias); rows are written by off-critical-path
  SBUF DMAs one step behind, with rank-1 matmul corrections for the last two
  rows. The softmax-weighted bias-v contribution is exactly b_v, folded into
  the out-projection bias on the host.
"""

import os
import numpy as np

import concourse.bass as bass
import concourse.mybir as mybir
from concourse import bacc
from concourse.tile import TileContext

F32 = mybir.dt.float32
F32R = mybir.dt.float32r
def _r(ap):
    return ap.bitcast(F32R)
AF = mybir.ActivationFunctionType
ALU = mybir.AluOpType

B, T, DIN, DOUT = 32, 64, 512, 128
NHEAD = 8
FF_ENC, FF_DEC, NLAYERS = 64, 16, 4
EPS = 1e-5
NCORES = 8
BL = B // NCORES          # local batch = 4
NTOK = BL * T             # 256 local encoder tokens
DHE = DIN // NHEAD        # 64 encoder head dim
DHD = DOUT // NHEAD       # 16 decoder head dim
NSTEP = int(os.environ.get("KERNEL_NSTEP", T))
NENC = int(os.environ.get("KERNEL_NENC", NLAYERS))

_CACHE = {}


def _patch_act_table_pass(nc):
    """All activation funcs we use (Exp, Ln, Square, Relu, Identity, Copy) live
    in the combined natural_log_exp_and_others table, but the auto-inserted
    loads alternate between the exp-only and ln-only sets (~1.3us each).
    Make every other set look empty so the insertion pass maps all
    activations to the combined set and hoists to a single load."""
    import types
    import bass_rust as _br
    from concourse.hw_specs import get_activation_tables

    def patched(self):
        has_activation = any(
            isinstance(i, mybir.InstActivation)
            for b in self.main_func.blocks
            for i in b.instructions
        )
        if not has_activation:
            return
        tabs = get_activation_tables(self.m.arch)
        keep = "natural_log_exp_and_others"
        for f in self.m.functions:
            for blk in f.blocks:
                for ins in blk.instructions:
                    if isinstance(ins, mybir.InstActivation):
                        assert ins.func in tabs[keep], f"{ins.func} not in {keep}"
        tables = [(k, (v if k == keep else set())) for k, v in tabs.items()]
        _br.insert_act_table_loads(self, tables)

    nc.insert_act_table_loads = types.MethodType(patched, nc)


def _split_drain_waits(nc, maxw=1):
    """Walrus in this container rejects >1 sync-wait on CTRL-class (Drain)
    instructions; split extras onto preceding nops on the same engine."""
    n = 0
    for f in nc.m.functions:
        for blk in f.blocks:
            newlist = []
            for ins in blk.instructions:
                si = ins.sync_info
                if si is not None and len(si.on_wait) > maxw and type(ins).__name__ == "InstDrain":
                    waits = list(si.on_wait)
                    for w in waits[:-maxw]:
                        nop = mybir.InstNoOp(name=f"Wsplit{n}", ins=[], outs=[])
                        n += 1
                        nop.engine = ins.engine
                        nop.sync_info = mybir.SyncInfo(on_wait=[w], on_update=[])
                        newlist.append(nop)
                    ins.sync_info = mybir.SyncInfo(on_wait=waits[-maxw:], on_update=list(si.on_update))
                newlist.append(ins)
            blk.instructions = newlist


def build_program():
    nc = bacc.Bacc("TRN2", target_bir_lowering=False, debug=False)
    D = {}

    def din(name, shape):
        D[name] = nc.dram_tensor(name, list(shape), F32, kind="ExternalInput").ap()
        return D[name]

    din("src_tok", [NTOK, DIN])
    din("srcT", [DIN, NTOK])
    for l in range(NLAYERS):
        din(f"e{l}_qkvT", [DIN, 3 * DIN])
        din(f"e{l}_bqk", [128, 8])
        din(f"e{l}_bv", [1, DIN])
        din(f"e{l}_woT", [DIN, DIN])
        din(f"e{l}_bo", [1, DIN])
        din(f"e{l}_f1T", [DIN, FF_ENC])
        din(f"e{l}_bf1", [FF_ENC, 1])
        din(f"e{l}_f2T", [FF_ENC, DIN])
        din(f"e{l}_bf2", [1, DIN])
        din(f"e{l}_g1", [1, DIN])
        din(f"e{l}_b1", [1, DIN])
        din(f"e{l}_g2", [1, DIN])
        din(f"e{l}_b2", [1, DIN])
    din("fcT", [DIN, DOUT])
    din("bfc", [1, DOUT])
    din("d_sqT", [DOUT, DOUT]); din("d_bsq", [DOUT, 1])
    din("d_skT", [DOUT, DOUT]); din("d_bsk", [DOUT, 1])
    din("d_svT", [DOUT, DOUT])
    din("d_soT", [DOUT, DOUT]); din("d_bso", [1, DOUT])
    din("d_cqT", [DOUT, DOUT]); din("d_bcq", [DOUT, 1])
    din("d_ckT", [DOUT, DOUT]); din("d_bck", [DOUT, 1])
    din("d_cvT", [DOUT, DOUT]); din("d_bcv", [1, DOUT])
    din("d_coT", [DOUT, DOUT]); din("d_bco", [1, DOUT])
    din("d_f1T", [DOUT, FF_DEC]); din("d_bf1", [FF_DEC, 1])
    din("d_f2T", [FF_DEC, DOUT]); din("d_bf2", [1, DOUT])
    for nm in ("g1", "b1", "g2", "b2", "g3", "b3"):
        din(f"d_{nm}c", [DOUT, 1])
    din("identity", [128, 128])
    din("mask", [128, NHEAD])
    din("maskT", [NHEAD, 128])
    din("ones", [128, 1])

    out_d = nc.dram_tensor("out", [BL, T, DOUT], F32, kind="ExternalOutput").ap()

    with TileContext(nc) as tc:
        _build_body(nc, tc, D, out_d)

    _patch_act_table_pass(nc)
    nc.compile()
    _split_drain_waits(nc)
    return nc, list(D.keys())


def _ln_tokmajor(nc, pool, pre, nparts, dfeat, g_b, b_b, out_ap, eps_ap, eng2=None):
    """LayerNorm over the free dim of token-major `pre` [nparts, dfeat]."""
    ve = nc.vector
    e2 = eng2 or ve
    s1 = pool.tile([nparts, 1], F32, tag="ln_s1")
    ve.tensor_reduce(out=s1[:], in_=pre, op=ALU.add, axis=mybir.AxisListType.X)
    mu = pool.tile([nparts, 1], F32, tag="ln_mu")
    ve.tensor_scalar_mul(mu[:], s1[:], 1.0 / dfeat)
    sqj = pool.tile([nparts, dfeat], F32, tag="ln_sqj")
    s2 = pool.tile([nparts, 1], F32, tag="ln_s2")
    nc.scalar.activation(sqj[:], pre, AF.Square, accum_out=s2[:])
    mu2 = pool.tile([nparts, 1], F32, tag="ln_mu2")
    ve.tensor_mul(mu2[:], mu[:], mu[:])
    var = pool.tile([nparts, 1], F32, tag="ln_var")
    ve.tensor_scalar(var[:], s2[:], 1.0 / dfeat, mu2[:], op0=ALU.mult, op1=ALU.subtract)
    # rstd = exp(-0.5*ln(var+eps)): keeps ACT in the natural_log_exp func set
    lnv = pool.tile([nparts, 1], F32, tag="ln_lnv")
    nc.scalar.activation(lnv[:], var[:], AF.Ln, bias=eps_ap)
    al = pool.tile([nparts, 1], F32, tag="ln_al")
    nc.scalar.activation(al[:], lnv[:], AF.Exp, scale=-0.5)
    mup = pool.tile([nparts, 1], F32, tag="ln_mup")
    ve.tensor_scalar(mup[:], mu[:], al[:], -1.0, op0=ALU.mult, op1=ALU.mult)
    xn = pool.tile([nparts, dfeat], F32, tag="ln_xn")
    ve.tensor_scalar(xn[:], pre, al[:], mup[:], op0=ALU.mult, op1=ALU.add)
    xg = pool.tile([nparts, dfeat], F32, tag="ln_xg")
    ve.tensor_mul(xg[:], xn[:], g_b)
    e2.tensor_add(out_ap, xg[:], b_b)
    return out_ap


def _build_body(nc, tc, D, out_d):
    import contextlib
    ctx = contextlib.ExitStack()
    ectx = contextlib.ExitStack()
    with ctx:
        cpool = ctx.enter_context(tc.tile_pool(name="const", bufs=1))
        w2pool = ectx.enter_context(tc.tile_pool(name="wts2", bufs=2))
        w1pool = ectx.enter_context(tc.tile_pool(name="wts1", bufs=1))
        apool = ectx.enter_context(tc.tile_pool(name="acts", bufs=1))
        spool = ectx.enter_context(tc.tile_pool(name="small", bufs=3))

        ident = cpool.tile([128, 128], F32, tag="ident")
        nc.sync.dma_start(out=ident[:], in_=D["identity"])
        mask = cpool.tile([128, NHEAD], F32, tag="mask")
        nc.sync.dma_start(out=mask[:], in_=D["mask"])
        maskT = cpool.tile([NHEAD, 128], F32, tag="maskT")
        nc.sync.dma_start(out=maskT[:], in_=D["maskT"])
        ones = cpool.tile([128, 1], F32, tag="ones_t")
        nc.sync.dma_start(out=ones[:], in_=D["ones"])
        eps_t = cpool.tile([128, 1], F32, tag="eps_t")
        nc.vector.memset(eps_t[:], EPS)

        # ---------------- encoder ----------------
        X_tok, XT = [], []
        for tt in range(2):
            xt_ = apool.tile([128, DIN], F32, tag=f"X_tok{tt}")
            nc.sync.dma_start(out=xt_[:], in_=D["src_tok"][tt * 128:(tt + 1) * 128, :])
            X_tok.append(xt_[:])
        for c in range(4):
            xc = apool.tile([128, NTOK], F32, tag=f"XT{c}")
            nc.sync.dma_start(out=xc[:], in_=D["srcT"][c * 128:(c + 1) * 128, :])
            XT.append(xc[:])

        for l in range(NENC):
            X_tok, XT = _enc_layer(nc, tc, D, l, X_tok, XT,
                                   w2pool, w1pool, apool, spool, ident, eps_t)

        # ---------------- fc + memory K/V ----------------
        fcTs = []
        for c in range(4):
            t_ = w1pool.tile([128, DOUT], F32, tag=f"fcT{c}")
            nc.sync.dma_start(out=t_[:], in_=D["fcT"][c * 128:(c + 1) * 128, :])
            fcTs.append(t_)
        bfc_b = cpool.tile([128, DOUT], F32, tag="bfc_b")
        _bcast_row(nc, cpool, D["bfc"], bfc_b, 128, "bfc")

        ckT = cpool.tile([DOUT, DOUT], F32, tag="d_ckT")
        nc.sync.dma_start(out=ckT[:], in_=D["d_ckT"])
        bck = cpool.tile([DOUT, 1], F32, tag="d_bck")
        nc.sync.dma_start(out=bck[:], in_=D["d_bck"])
        cvT = cpool.tile([DOUT, DOUT], F32, tag="d_cvT")
        nc.sync.dma_start(out=cvT[:], in_=D["d_cvT"])
        bcv_b = cpool.tile([128, DOUT], F32, tag="bcv_b")
        _bcast_row(nc, cpool, D["d_bcv"], bcv_b, 128, "bcv")

        Kmem = cpool.tile([128, NTOK], F32, tag="Kmem")
        Vmem = [cpool.tile([T, DOUT], F32, tag=f"Vmem{b}", name=f"Vmem{b}") for b in range(BL)]
        with tc.tile_pool(name="psfc", bufs=2, space="PSUM") as psfc:
            mem_tok = []
            for tt in range(2):
                mp = psfc.tile([128, DOUT], F32, tag="mem")
                for c in range(4):
                    nc.tensor.matmul(mp[:], XT[c][:, tt * 128:(tt + 1) * 128], fcTs[c][:],
                                     start=(c == 0), stop=(c == 3))
                ms = apool.tile([128, DOUT], F32, tag=f"mem_tok{tt}")
                nc.vector.tensor_add(ms[:], mp[:], bfc_b[:])
                mem_tok.append(ms)
            memT = apool.tile([128, NTOK], F32, tag="memT")
            for tt in range(2):
                tp = psfc.tile([128, 128], F32, tag="memTp")
                nc.tensor.transpose(tp[:], mem_tok[tt][:], ident[:])
                nc.scalar.copy(memT[:, tt * 128:(tt + 1) * 128], tp[:])
            kmp = psfc.tile([128, NTOK], F32, tag="kmem")
            nc.tensor.matmul(kmp[:], ckT[:], memT[:], start=True, stop=True)
            nc.scalar.activation(Kmem[:], kmp[:], AF.Identity, bias=bck[:])
            for b in range(BL):
                vmp = psfc.tile([T, DOUT], F32, tag="vmem")
                nc.tensor.matmul(vmp[:], memT[:, b * T:(b + 1) * T], cvT[:],
                                 start=True, stop=True)
                nc.vector.tensor_add(Vmem[b][:], vmp[:], bcv_b[0:T, :])

        # ---------------- decoder prep ----------------
        dw = {}
        for nm in ("d_sqT", "d_skT", "d_svT", "d_soT", "d_cqT", "d_coT"):
            t_ = cpool.tile([DOUT, DOUT], F32, tag=nm)
            nc.sync.dma_start(out=t_[:], in_=D[nm])
            dw[nm] = t_
        d_f1T = cpool.tile([DOUT, FF_DEC], F32, tag="d_f1T")
        nc.sync.dma_start(out=d_f1T[:], in_=D["d_f1T"])
        d_f2T = cpool.tile([FF_DEC, DOUT], F32, tag="d_f2T")
        nc.sync.dma_start(out=d_f2T[:], in_=D["d_f2T"])
        bsq = cpool.tile([DOUT, 1], F32, tag="d_bsq")
        nc.sync.dma_start(out=bsq[:], in_=D["d_bsq"])
        bsk = cpool.tile([DOUT, 1], F32, tag="d_bsk")
        nc.sync.dma_start(out=bsk[:], in_=D["d_bsk"])
        bcq = cpool.tile([DOUT, 1], F32, tag="d_bcq")
        nc.sync.dma_start(out=bcq[:], in_=D["d_bcq"])
        d_bf1 = cpool.tile([FF_DEC, 1], F32, tag="d_bf1")
        nc.sync.dma_start(out=d_bf1[:], in_=D["d_bf1"])
        gb = {}
        for nm in ("g1", "b1", "g2", "b2", "g3", "b3"):
            t_ = cpool.tile([DOUT, 1], F32, tag=f"c_{nm}")
            nc.sync.dma_start(out=t_[:], in_=D[f"d_{nm}c"])
            gb[nm] = t_
        rows = {}
        for nm in ("d_bso", "d_bco", "d_bf2"):
            r_ = cpool.tile([1, DOUT], F32, tag=f"row_{nm}")
            nc.sync.dma_start(out=r_[:], in_=D[nm])
            rows[nm] = r_
        ones_r = cpool.tile([1, 128], F32, tag="ones_r")
        nc.vector.memset(ones_r[:], 1.0)
        rows["ones_r"] = ones_r

        # K cache prefilled with k-bias (k of zero rows); V-delta cache zero.
        Kc = cpool.tile([128, BL * (T + 1)], F32, tag="Kc")
        nc.vector.tensor_copy(Kc[:], bsk[:].broadcast_to([128, BL * (T + 1)]))
        Vdc = cpool.tile([T + 1, BL * DOUT], F32, tag="Vdc")
        nc.vector.memset(Vdc[:], 0.0)

        ectx.close()   # release encoder-phase SBUF before the decode loop
        opool = ctx.enter_context(tc.tile_pool(name="outp", bufs=1))
        out_sb = opool.tile([BL, T * DOUT], F32, tag="out_sb")
        if NSTEP < T:
            nc.vector.memset(out_sb[:], 0.0)
        zeroT = cpool.tile([DOUT, BL], F32, tag="zeroT")
        nc.vector.memset(zeroT[:], 0.0)

        # ---------------- decode loop ----------------
        with tc.tile_pool(name="dstep", bufs=3) as dpool, \
             tc.tile_pool(name="psD", bufs=1, space="PSUM") as psD:
            xT = zeroT[:]
            vd_prev = None
            for t in range(NSTEP):
                xT, vd_prev = _dec_step(nc, t, xT, vd_prev, Kc, Vdc, Kmem, Vmem,
                                        dw, bsq, bsk, bcq, d_f1T, d_f2T, d_bf1,
                                        gb, rows, mask, maskT, ones, ident,
                                        dpool, psD, out_sb)

        nc.sync.dma_start(out=out_d.rearrange("b t d -> b (t d)"), in_=out_sb[:])


def _bcast_row(nc, cpool, dram_row, dst_tile, channels, key):
    row = cpool.tile([1, dram_row.shape[-1]], F32, tag=f"brow_{key}")
    nc.sync.dma_start(out=row[:], in_=dram_row)
    nc.gpsimd.partition_broadcast(dst_tile[:], row[:], channels=channels)


def _enc_layer(nc, tc, D, l, X_tok, XT, w2pool, w1pool, apool, spool, ident, eps_t):
    qkvT = []
    for c in range(4):
        t_ = w2pool.tile([128, 3 * DIN], F32, tag=f"qkvT{c}")
        nc.sync.dma_start(out=t_[:], in_=D[f"e{l}_qkvT"][c * 128:(c + 1) * 128, :])
        qkvT.append(t_)
    woT = []
    for c in range(4):
        t_ = w1pool.tile([128, DIN], F32, tag=f"woT{c}")
        nc.sync.dma_start(out=t_[:], in_=D[f"e{l}_woT"][c * 128:(c + 1) * 128, :])
        woT.append(t_)
    f1T = []
    for c in range(4):
        t_ = w1pool.tile([128, FF_ENC], F32, tag=f"f1T{c}")
        nc.sync.dma_start(out=t_[:], in_=D[f"e{l}_f1T"][c * 128:(c + 1) * 128, :])
        f1T.append(t_)
    f2T = w1pool.tile([FF_ENC, DIN], F32, tag="f2T")
    nc.sync.dma_start(out=f2T[:], in_=D[f"e{l}_f2T"])
    bqk = w1pool.tile([128, 8], F32, tag="bqk")
    nc.sync.dma_start(out=bqk[:], in_=D[f"e{l}_bqk"])
    bf1 = w1pool.tile([FF_ENC, 1], F32, tag="bf1")
    nc.sync.dma_start(out=bf1[:], in_=D[f"e{l}_bf1"])
    bb = {}
    for nm in ("bv", "bo", "bf2", "g1", "b1", "g2", "b2"):
        b_ = w1pool.tile([128, DIN], F32, tag=f"bb_{nm}")
        row = w1pool.tile([1, DIN], F32, tag=f"bbrow_{nm}")
        nc.sync.dma_start(out=row[:], in_=D[f"e{l}_{nm}"])
        nc.gpsimd.partition_broadcast(b_[:], row[:], channels=128)
        bb[nm] = b_

    # --- QKV ---
    QK = []
    V = []
    with tc.tile_pool(name=f"ps_qkv{l}", bufs=2, space="PSUM") as psq:
        for m in range(8):
            pq = psq.tile([128, NTOK], F32, tag="qk")
            for c in range(4):
                nc.tensor.matmul(pq[:], qkvT[c][:, m * 128:(m + 1) * 128], XT[c],
                                 start=(c == 0), stop=(c == 3))
            qs = apool.tile([128, NTOK], F32, tag=f"QK{m}")
            nc.scalar.activation(qs[:], pq[:], AF.Identity, bias=bqk[:, m:m + 1])
            QK.append(qs)
        for b in range(BL):
            pv = psq.tile([T, DIN], F32, tag="v")
            for c in range(4):
                nc.tensor.matmul(pv[:], XT[c][:, b * T:(b + 1) * T],
                                 qkvT[c][:, 2 * DIN:3 * DIN],
                                 start=(c == 0), stop=(c == 3))
            vs = apool.tile([T, DIN], F32, tag=f"V{b}", name=f"Vb{b}")
            nc.vector.tensor_add(vs[:], pv[:], bb["bv"][0:T, :])
            V.append(vs)

    # --- attention ---
    OT = [apool.tile([128, NTOK], F32, tag=f"OT{c}", name=f"OT{c}") for c in range(4)]
    with tc.tile_pool(name=f"ps_att{l}", bufs=2, space="PSUM") as psa, \
         tc.tile_pool(name=f"sb_att{l}", bufs=4) as sba:
        for b in range(BL):
            den = spool.tile([T, NHEAD], F32, tag="den")
            Ps = []
            for h in range(NHEAD):
                c, r0 = h // 2, (h % 2) * DHE
                Qs = QK[c][r0:r0 + DHE, b * T:(b + 1) * T]
                Ks = QK[4 + c][r0:r0 + DHE, b * T:(b + 1) * T]
                sp = psa.tile([T, T], F32, tag="S", bufs=3)
                nc.tensor.matmul(sp[:], Qs, Ks, start=True, stop=True)
                p_ = sba.tile([T, T], F32, tag="P", bufs=9)
                nc.scalar.activation(p_[:], sp[:], AF.Exp, scale=1.0 / np.sqrt(DHE),
                                     accum_out=den[:, h:h + 1])
                Ps.append(p_)
            rb = spool.tile([T, NHEAD], F32, tag="rb")
            nc.vector.reciprocal(rb[:], den[:])
            for h in range(NHEAD):
                c, r0 = h // 2, (h % 2) * DHE
                a_ = sba.tile([T, T], F32, tag="A")
                nc.scalar.activation(a_[:], Ps[h][:], AF.Copy, scale=rb[:, h:h + 1])
                atp = psa.tile([T, T], F32, tag="AT")
                nc.tensor.transpose(atp[:], a_[:], ident[0:T, 0:T])
                ats = sba.tile([T, T], F32, tag="ATs")
                nc.vector.tensor_copy(ats[:], atp[:])
                avp = psa.tile([DHE, T], F32, tag="AV")
                Vs = V[b][0:T, h * DHE:(h + 1) * DHE]
                nc.tensor.matmul(avp[:], Vs, ats[:], start=True, stop=True)
                nc.scalar.copy(OT[c][r0:r0 + DHE, b * T:(b + 1) * T], avp[:])

    # --- out-proj + residual + LN1 ---
    X1_tok = []
    X1T = [apool.tile([128, NTOK], F32, tag=f"X1T{c}", name=f"X1T{c}") for c in range(4)]
    with tc.tile_pool(name=f"ps_o{l}", bufs=2, space="PSUM") as pso:
        for tt in range(2):
            ap_ = pso.tile([128, DIN], F32, tag="ao")
            for c in range(4):
                nc.tensor.matmul(ap_[:], OT[c][:, tt * 128:(tt + 1) * 128], woT[c][:],
                                 start=(c == 0), stop=(c == 3))
            t1 = apool.tile([128, DIN], F32, tag=f"pre1_{tt}")
            nc.vector.tensor_add(t1[:], ap_[:], X_tok[tt])
            nc.vector.tensor_add(t1[:], t1[:], bb["bo"][:])
            x1 = apool.tile([128, DIN], F32, tag=f"X1_{tt}")
            _ln_tokmajor(nc, spool, t1[:], 128, DIN, bb["g1"][:], bb["b1"][:], x1[:], eps_t[:])
            X1_tok.append(x1[:])
        for tt in range(2):
            for c in range(4):
                tp = pso.tile([128, 128], F32, tag="xT")
                nc.tensor.transpose(tp[:], X1_tok[tt][:, c * 128:(c + 1) * 128], ident[:])
                nc.scalar.copy(X1T[c][:, tt * 128:(tt + 1) * 128], tp[:])

    # --- FFN + LN2 ---
    X2_tok = []
    X2T = [apool.tile([128, NTOK], F32, tag=f"X2T{c}", name=f"X2T{c}") for c in range(4)]
    with tc.tile_pool(name=f"ps_f{l}", bufs=2, space="PSUM") as psf:
        hp = psf.tile([FF_ENC, NTOK], F32, tag="h")
        for c in range(4):
            nc.tensor.matmul(hp[:], f1T[c][:], X1T[c][:], start=(c == 0), stop=(c == 3))
        hs = apool.tile([FF_ENC, NTOK], F32, tag="H")
        nc.scalar.activation(hs[:], hp[:], AF.Relu, bias=bf1[:])
        for tt in range(2):
            fp = psf.tile([128, DIN], F32, tag="f")
            nc.tensor.matmul(fp[:], hs[:, tt * 128:(tt + 1) * 128], f2T[:],
                             start=True, stop=True)
            t2 = apool.tile([128, DIN], F32, tag=f"pre2_{tt}")
            nc.vector.tensor_add(t2[:], fp[:], X1_tok[tt])
            nc.vector.tensor_add(t2[:], t2[:], bb["bf2"][:])
            x2 = apool.tile([128, DIN], F32, tag=f"X2_{tt}")
            _ln_tokmajor(nc, spool, t2[:], 128, DIN, bb["g2"][:], bb["b2"][:], x2[:], eps_t[:])
            X2_tok.append(x2[:])
        for tt in range(2):
            for c in range(4):
                tp = psf.tile([128, 128], F32, tag="xT2")
                nc.tensor.transpose(tp[:], X2_tok[tt][:, c * 128:(c + 1) * 128], ident[:])
                nc.scalar.copy(X2T[c][:, tt * 128:(tt + 1) * 128], tp[:])

    return X2_tok, X2T


def _attn_fm(nc, t, xT_ap, dpool, bank, ppb, qT, bq, K_ap, vlen, V_list, corr,
             mask, maskT, ones, brow_o, oT, ident, ones1b, pfx, scale):
    """Feature-major decoder attention sublayer.

    Returns the pre-LN residual PSUM tile AP [128, BL] (residual + bias +
    out-proj accumulated; caller copies to SBUF and layernorms).
    corr: list of (vd_rows: [4 x AP[1,128]], kcol: fn(b)->AP[128,1]) rank-1 V
    corrections; the exp'd score row is recomputed from the cache column.
    bank: a [128, 512] PSUM tile carved into column slices for this sublayer.
    ppb: a dedicated [128, BL] PSUM tile for the long-lived residual
    accumulation (CoreSim forbids reading a tensor with an open accum group).
    """
    qp = bank[0:DOUT, 0:BL]
    nc.tensor.matmul(qp, qT[:], xT_ap, start=True, stop=True)
    # residual accumulation starts early on its own PSUM tile
    pp = ppb[0:128, 0:BL]
    nc.tensor.matmul(pp, ident[:], xT_ap, start=True, stop=False)
    nc.tensor.matmul(pp, brow_o, ones1b, start=False, stop=False)
    # qblk = (q + bq) * head-mask, fused
    qblk = dpool.tile([128, BL * NHEAD], F32, tag=f"{pfx}qblk")
    nc.vector.scalar_tensor_tensor(
        out=qblk[:].rearrange("p (b h) -> p b h", b=BL),
        in0=qp.unsqueeze(2).broadcast_to([128, BL, NHEAD]),
        scalar=bq[:],
        in1=mask[:].unsqueeze(1).broadcast_to([128, BL, NHEAD]),
        op0=ALU.add, op1=ALU.mult)
    stp = bank[0:vlen, 8:8 + BL * NHEAD]
    for b in range(BL):
        nc.tensor.matmul(stp[:, b * NHEAD:(b + 1) * NHEAD],
                         K_ap[:, b * vlen:(b + 1) * vlen],
                         qblk[:, b * NHEAD:(b + 1) * NHEAD], start=True, stop=True)
    pt = dpool.tile([vlen, BL * NHEAD], F32, tag=f"{pfx}pt")
    nc.scalar.activation(pt[:], stp, AF.Exp, scale=scale)
    # exp'd score rows for the stale cache row (parallel to the main path)
    ptr = []
    for ci, (vdr, kcol) in enumerate(corr):
        srow = bank[0:1, 40 + 32 * ci:72 + 32 * ci]
        for b in range(BL):
            bsl = slice(b * NHEAD, (b + 1) * NHEAD)
            nc.tensor.matmul(srow[:, bsl], kcol(b), qblk[:, bsl],
                             start=True, stop=True)
        pr = dpool.tile([1, BL * NHEAD], F32, tag=f"{pfx}pr{ci}")
        nc.scalar.activation(pr[:], srow, AF.Exp, scale=scale)
        ptr.append(pr)
    denp = bank[0:NHEAD, 72:72 + BL]
    avp = bank[0:128, 76:76 + BL * NHEAD]
    for b in range(BL):
        bsl = slice(b * NHEAD, (b + 1) * NHEAD)
        nc.tensor.matmul(denp[:, b:b + 1], pt[:, bsl], ones[0:vlen, :],
                         start=True, stop=True)
        nc.tensor.matmul(avp[:, bsl], V_list[b], pt[:, bsl],
                         start=True, stop=(len(corr) == 0))
        for ci, (vdr, kcol) in enumerate(corr):
            nc.tensor.matmul(avp[:, bsl], vdr[b][:], ptr[ci][0:1, bsl],
                             start=False, stop=(ci == len(corr) - 1))
    r_ = dpool.tile([NHEAD, BL], F32, tag=f"{pfx}r")
    nc.vector.reciprocal(r_[:], denp)
    avm = dpool.tile([128, BL * NHEAD], F32, tag=f"{pfx}avm")
    nc.vector.tensor_mul(
        avm[:].rearrange("p (b h) -> p b h", b=BL),
        avp.rearrange("p (b h) -> p b h", b=BL),
        mask[:].unsqueeze(1).broadcast_to([128, BL, NHEAD]))
    o_ = dpool.tile([128, BL], F32, tag=f"{pfx}o")
    nc.vector.tensor_reduce(out=o_[:], in_=avm[:].rearrange("p (b h) -> p b h", b=BL),
                            op=ALU.add, axis=mybir.AxisListType.X)
    erp = bank[0:128, 108:108 + BL]
    nc.tensor.matmul(erp, maskT[:], r_[:], start=True, stop=True)
    on = dpool.tile([128, BL], F32, tag=f"{pfx}on")
    nc.vector.tensor_mul(on[:], o_[:], erp)
    nc.tensor.matmul(pp, oT[:], on[:], start=False, stop=True)
    return pp


def _ln_fm(nc, t, dpool, psD, pp, g_c, b_c, pfx):
    """Feature-major LN: copy PSUM->SBUF then fused gpsimd layernorm per col."""
    ps = dpool.tile([128, BL], F32, tag=f"{pfx}ps")
    nc.vector.tensor_copy(ps[:], pp[:])
    xo = dpool.tile([128, BL], F32, tag=f"{pfx}xo")
    for b in range(BL):
        nc.gpsimd.layernorm(out_ap=xo[:, b:b + 1], in_ap=ps[:, b:b + 1],
                            gamma_ap=g_c[:], beta_ap=b_c[:], eps=EPS,
                            subtract_mean=True, n_tokens=1)
    return xo


def _dec_step(nc, t, xT, vd_prev, Kc, Vdc, Kmem, Vmem, dw, bsq, bsk, bcq,
              d_f1T, d_f2T, d_bf1, gb, rows, mask, maskT, ones, ident,
              dpool, psD, out_sb):
    ones1b = rows["ones_r"][0:1, 0:BL]
    saB = psD.tile([128, 512], F32, tag="saB")
    caB = psD.tile([128, 512], F32, tag="caB")
    saP = psD.tile([128, BL], F32, tag="saP")
    caP = psD.tile([128, BL], F32, tag="caP")
    ffP = psD.tile([128, BL], F32, tag="ffP")
    mscB = psD.tile([128, 512], F32, tag="mscB")
    xtrB = psD.tile([BL, DOUT], F32, tag="xtrB")
    vrB = psD.tile([1, 512], F32, tag="vrB")

    # --- k/v projections into caches ---
    kp = mscB[0:DOUT, 0:BL]
    nc.tensor.matmul(kp, dw["d_skT"][:], xT, start=True, stop=True)
    kslice = Kc[:].rearrange("p (b j) -> p b j", b=BL)[:, :, t]
    nc.vector.tensor_scalar_add(kslice, kp, bsk[:])
    # v-delta rows (no bias), one [1,128] row per batch (matmul operands must
    # sit at partition 0), used for the rank-1 row-t correction + cache DMA
    vd = []
    for b in range(BL):
        vrp = vrB[0:1, b * DOUT:(b + 1) * DOUT]
        nc.tensor.matmul(vrp, xT[:, b:b + 1], dw["d_svT"][:],
                         start=True, stop=True)
        vr = dpool.tile([1, DOUT], F32, tag=f"vd{b}")
        nc.scalar.copy(vr[:], vrp)
        vd.append(vr)

    corr = [(vd, lambda b: Kc[:, b * (T + 1) + t:b * (T + 1) + t + 1])]
    Vd_list = [Vdc[:, b * DOUT:(b + 1) * DOUT] for b in range(BL)]
    pre1 = _attn_fm(nc, t, xT, dpool, saB, saP, dw["d_sqT"], bsq, Kc[:], T + 1,
                    Vd_list, corr, mask, maskT, ones, rows["d_bso"][:],
                    dw["d_soT"], ident, ones1b, "sa", 1.0 / np.sqrt(DHD))
    # cache row write for future steps (after this step's reads, off-path;
    # tile's RAW dep makes step t+1's AV wait for this DMA, which lands well
    # within one step's slack)
    for b in range(BL):
        nc.sync.dma_start(out=Vdc[t:t + 1, b * DOUT:(b + 1) * DOUT],
                          in_=vd[b][:])
    x1T = _ln_fm(nc, t, dpool, psD, pre1, gb["g1"], gb["b1"], "l1")

    Vm_list = [Vmem[b][:] for b in range(BL)]
    pre2 = _attn_fm(nc, t, x1T[:], dpool, caB, caP, dw["d_cqT"], bcq, Kmem[:], T,
                    Vm_list, [], mask, maskT, ones, rows["d_bco"][:],
                    dw["d_coT"], ident, ones1b, "ca", 1.0 / np.sqrt(DHD))
    x2T = _ln_fm(nc, t, dpool, psD, pre2, gb["g2"], gb["b2"], "l2")

    # --- FFN ---
    hp = mscB[0:FF_DEC, BL:2 * BL]
    nc.tensor.matmul(hp, d_f1T[:], x2T[:], start=True, stop=True)
    h_ = dpool.tile([FF_DEC, BL], F32, tag="hdec")
    nc.scalar.activation(h_[:], hp, AF.Relu, bias=d_bf1[:])
    pre3 = ffP[0:128, 0:BL]
    nc.tensor.matmul(pre3, ident[:], x2T[:], start=True, stop=False)
    nc.tensor.matmul(pre3, rows["d_bf2"][:], ones1b, start=False, stop=False)
    nc.tensor.matmul(pre3, d_f2T[:], h_[:], start=False, stop=True)
    xoT = _ln_fm(nc, t, dpool, psD, pre3, gb["g3"], gb["b3"], "l3")

    # --- emit output row (off critical path) ---
    xtr = xtrB[0:BL, 0:DOUT]
    nc.tensor.transpose(xtr, xoT[:], ident[:])
    nc.scalar.copy(out_sb[:, t * DOUT:(t + 1) * DOUT], xtr)
    return xoT[:], None


# ------------------------------------------------------------------
# host side
# ------------------------------------------------------------------

def _prep_shared(inputs):
    f = np.ascontiguousarray
    S = {}
    for l in range(NLAYERS):
        qkv_w = inputs["enc_qkv_w"][l]
        S[f"e{l}_qkvT"] = f(qkv_w.T)
        qkv_b = inputs["enc_qkv_b"][l]
        S[f"e{l}_bqk"] = f(qkv_b[:2 * DIN].reshape(8, 128).T)
        S[f"e{l}_bv"] = f(qkv_b[2 * DIN:].reshape(1, DIN))
        S[f"e{l}_woT"] = f(inputs["enc_out_w"][l].T)
        S[f"e{l}_bo"] = f(inputs["enc_out_b"][l].reshape(1, DIN))
        S[f"e{l}_f1T"] = f(inputs["enc_ff1_w"][l].T)
        S[f"e{l}_bf1"] = f(inputs["enc_ff1_b"][l].reshape(FF_ENC, 1))
        S[f"e{l}_f2T"] = f(inputs["enc_ff2_w"][l].T)
        S[f"e{l}_bf2"] = f(inputs["enc_ff2_b"][l].reshape(1, DIN))
        S[f"e{l}_g1"] = f(inputs["enc_ln1_g"][l].reshape(1, DIN))
        S[f"e{l}_b1"] = f(inputs["enc_ln1_b"][l].reshape(1, DIN))
        S[f"e{l}_g2"] = f(inputs["enc_ln2_g"][l].reshape(1, DIN))
        S[f"e{l}_b2"] = f(inputs["enc_ln2_b"][l].reshape(1, DIN))
    S["fcT"] = f(inputs["fc_w"].T)
    S["bfc"] = f(inputs["fc_b"].reshape(1, DOUT))
    sq, sk, sv = np.split(inputs["dec_sa_qkv_w"], 3, axis=0)
    bq_, bk_, bv_ = np.split(inputs["dec_sa_qkv_b"], 3)
    S["d_sqT"] = f(sq.T); S["d_bsq"] = f(bq_.reshape(DOUT, 1))
    S["d_skT"] = f(sk.T); S["d_bsk"] = f(bk_.reshape(DOUT, 1))
    S["d_svT"] = f(sv.T)
    S["d_soT"] = f(inputs["dec_sa_out_w"].T)
    # softmax weights sum to 1, so the v-bias contributes exactly b_v to the
    # attention output; fold W_o @ b_v into the out-proj bias.
    S["d_bso"] = f((inputs["dec_sa_out_b"] + inputs["dec_sa_out_w"] @ bv_).reshape(1, DOUT))
    cq, ck, cv = np.split(inputs["dec_ca_qkv_w"], 3, axis=0)
    cbq, cbk, cbv = np.split(inputs["dec_ca_qkv_b"], 3)
    S["d_cqT"] = f(cq.T); S["d_bcq"] = f(cbq.reshape(DOUT, 1))
    S["d_ckT"] = f(ck.T); S["d_bck"] = f(cbk.reshape(DOUT, 1))
    S["d_cvT"] = f(cv.T); S["d_bcv"] = f(cbv.reshape(1, DOUT))
    S["d_coT"] = f(inputs["dec_ca_out_w"].T)
    S["d_bco"] = f(inputs["dec_ca_out_b"].reshape(1, DOUT))
    S["d_f1T"] = f(inputs["dec_ff1_w"].T)
    S["d_bf1"] = f(inputs["dec_ff1_b"].reshape(FF_DEC, 1))
    S["d_f2T"] = f(inputs["dec_ff2_w"].T)
    S["d_bf2"] = f(inputs["dec_ff2_b"].reshape(1, DOUT))
    for nm in ("g1", "b1", "g2", "b2", "g3", "b3"):
        S[f"d_{nm}c"] = f(inputs[f"dec_ln{nm[1]}_{nm[0]}"].reshape(DOUT, 1))
    S["identity"] = np.eye(128, dtype=np.float32)
    S["mask"] = (np.arange(128)[:, None] // DHD == np.arange(NHEAD)[None, :]).astype(np.float32)
    S["maskT"] = f(S["mask"].T)
    S["ones"] = np.ones((128, 1), dtype=np.float32)
    return {k: np.asarray(v, dtype=np.float32) for k, v in S.items()}


def make_in_maps(inputs):
    shared = _prep_shared(inputs)
    src = np.asarray(inputs["src"], dtype=np.float32)
    in_maps = []
    for c in range(NCORES):
        shard = np.ascontiguousarray(src[c * BL:(c + 1) * BL])
        tok = shard.reshape(NTOK, DIN)
        m = dict(shared)
        m["src_tok"] = np.ascontiguousarray(tok)
        m["srcT"] = np.ascontiguousarray(tok.T)
        in_maps.append(m)
    return in_maps


def kernel(**inputs) -> np.ndarray:
    from concourse.bass_utils import run_bass_kernel_spmd
    if "nc" not in _CACHE:
        _CACHE["nc"] = build_program()[0]
    nc = _CACHE["nc"]
    in_maps = make_in_maps(inputs)
    res = run_bass_kernel_spmd(nc, in_maps, core_ids=list(range(NCORES)))
    out = np.concatenate([r["out"] for r in res.results], axis=0)
    return out.astype(np.float32)


# revision 14
# speedup vs baseline: 1.1862x; 1.0023x over previous
"""Trainium2 Bass kernel for nn_CustomTransformer_50062138802561.

4-layer encoder (d=512, 8 heads, ffn 64) + fc to 128 + 64-step sequential
decoder (single shared layer, d=128, 8 heads dh=16, ffn 16).

Strategy:
- Data-parallel over batch: 8 cores x 4 batches each. No collectives.
- Decoder loop rewritten as incremental KV-cache decode (mathematically
  identical to the reference's full-recompute loop: padded zero rows produce
  k=b_k / v=b_v which we pre-fill / fold into the out-proj bias).
- Decoder residual stream kept feature-major [128, BL]; residual adds,
  biases and projections all accumulate as PSUM matmuls, so no per-sublayer
  transposes.
- LayerNorm via the fused gpsimd partition-axis layernorm (one Pool op per
  batch column).
- V cache stores v-delta (v minus bias); rows are written by off-critical-path
  SBUF DMAs one step behind, with rank-1 matmul corrections for the last two
  rows. The softmax-weighted bias-v contribution is exactly b_v, folded into
  the out-projection bias on the host.
"""

import os
import numpy as np

import concourse.bass as bass
import concourse.mybir as mybir
from concourse import bacc
from concourse.tile import TileContext

F32 = mybir.dt.float32
F32R = mybir.dt.float32r
def _r(ap):
    return ap.bitcast(F32R)
AF = mybir.ActivationFunctionType
ALU = mybir.AluOpType

B, T, DIN, DOUT = 32, 64, 512, 128
NHEAD = 8
FF_ENC, FF_DEC, NLAYERS = 64, 16, 4
EPS = 1e-5
NCORES = 8
BL = B // NCORES          # local batch = 4
NTOK = BL * T             # 256 local encoder tokens
DHE = DIN // NHEAD        # 64 encoder head dim
DHD = DOUT // NHEAD       # 16 decoder head dim
NSTEP = int(os.environ.get("KERNEL_NSTEP", T))
NENC = int(os.environ.get("KERNEL_NENC", NLAYERS))

_CACHE = {}


def _patch_act_table_pass(nc):
    """All activation funcs we use (Exp, Ln, Square, Relu, Identity, Copy) live
    in the combined natural_log_exp_and_others table, but the auto-inserted
    loads alternate between the exp-only and ln-only sets (~1.3us each).
    Make every other set look empty so the insertion pass maps all
    activations to the combined set and hoists to a single load."""
    import types
    import bass_rust as _br
    from concourse.hw_specs import get_activation_tables

    def patched(self):
        has_activation = any(
            isinstance(i, mybir.InstActivation)
            for b in self.main_func.blocks
            for i in b.instructions
        )
        if not has_activation:
            return
        tabs = get_activation_tables(self.m.arch)
        keep = "natural_log_exp_and_others"
        for f in self.m.functions:
            for blk in f.blocks:
                for ins in blk.instructions:
                    if isinstance(ins, mybir.InstActivation):
                        assert ins.func in tabs[keep], f"{ins.func} not in {keep}"
        tables = [(k, (v if k == keep else set())) for k, v in tabs.items()]
        _br.insert_act_table_loads(self, tables)

    nc.insert_act_table_loads = types.MethodType(patched, nc)


def _split_drain_waits(nc, maxw=1):
    """Walrus in this container rejects >1 sync-wait on CTRL-class (Drain)
    instructions; split extras onto preceding nops on the same engine."""
    n = 0
    for f in nc.m.functions:
        for blk in f.blocks:
            newlist = []
            for ins in blk.instructions:
                si = ins.sync_info
                if si is not None and len(si.on_wait) > maxw and type(ins).__name__ == "InstDrain":
                    waits = list(si.on_wait)
                    for w in waits[:-maxw]:
                        nop = mybir.InstNoOp(name=f"Wsplit{n}", ins=[], outs=[])
                        n += 1
                        nop.engine = ins.engine
                        nop.sync_info = mybir.SyncInfo(on_wait=[w], on_update=[])
                        newlist.append(nop)
                    ins.sync_info = mybir.SyncInfo(on_wait=waits[-maxw:], on_update=list(si.on_update))
                newlist.append(ins)
            blk.instructions = newlist


def build_program():
    nc = bacc.Bacc("TRN2", target_bir_lowering=False, debug=False)
    D = {}

    def din(name, shape):
        D[name] = nc.dram_tensor(name, list(shape), F32, kind="ExternalInput").ap()
        return D[name]

    din("src_tok", [NTOK, DIN])
    din("srcT", [DIN, NTOK])
    for l in range(NLAYERS):
        din(f"e{l}_qkvT", [DIN, 3 * DIN])
        din(f"e{l}_bqk", [128, 8])
        din(f"e{l}_bv", [1, DIN])
        din(f"e{l}_woT", [DIN, DIN])
        din(f"e{l}_bo", [1, DIN])
        din(f"e{l}_f1T", [DIN, FF_ENC])
        din(f"e{l}_bf1", [FF_ENC, 1])
        din(f"e{l}_f2T", [FF_ENC, DIN])
        din(f"e{l}_bf2", [1, DIN])
        din(f"e{l}_g1", [1, DIN])
        din(f"e{l}_b1", [1, DIN])
        din(f"e{l}_g2", [1, DIN])
        din(f"e{l}_b2", [1, DIN])
    din("fcT", [DIN, DOUT])
    din("bfc", [1, DOUT])
    din("d_sqT", [DOUT, DOUT]); din("d_bsq", [DOUT, 1])
    din("d_skT", [DOUT, DOUT]); din("d_bsk", [DOUT, 1])
    din("d_svT", [DOUT, DOUT])
    din("d_soT", [DOUT, DOUT]); din("d_bso", [1, DOUT])
    din("d_cqT", [DOUT, DOUT]); din("d_bcq", [DOUT, 1])
    din("d_ckT", [DOUT, DOUT]); din("d_bck", [DOUT, 1])
    din("d_cvT", [DOUT, DOUT]); din("d_bcv", [1, DOUT])
    din("d_coT", [DOUT, DOUT]); din("d_bco", [1, DOUT])
    din("d_f1T", [DOUT, FF_DEC]); din("d_bf1", [FF_DEC, 1])
    din("d_f2T", [FF_DEC, DOUT]); din("d_bf2", [1, DOUT])
    for nm in ("g1", "b1", "g2", "b2", "g3", "b3"):
        din(f"d_{nm}c", [DOUT, 1])
    din("identity", [128, 128])
    din("mask", [128, NHEAD])
    din("maskT", [NHEAD, 128])
    din("ones", [128, 1])

    out_d = nc.dram_tensor("out", [BL, T, DOUT], F32, kind="ExternalOutput").ap()

    with TileContext(nc) as tc:
        _build_body(nc, tc, D, out_d)

    _patch_act_table_pass(nc)
    nc.compile()
    _split_drain_waits(nc)
    return nc, list(D.keys())


def _ln_tokmajor(nc, pool, pre, nparts, dfeat, g_b, b_b, out_ap, eps_ap, eng2=None):
    """LayerNorm over the free dim of token-major `pre` [nparts, dfeat]."""
    ve = nc.vector
    e2 = eng2 or ve
    s1 = pool.tile([nparts, 1], F32, tag="ln_s1")
    ve.tensor_reduce(out=s1[:], in_=pre, op=ALU.add, axis=mybir.AxisListType.X)
    mu = pool.tile([nparts, 1], F32, tag="ln_mu")
    ve.tensor_scalar_mul(mu[:], s1[:], 1.0 / dfeat)
    sqj = pool.tile([nparts, dfeat], F32, tag="ln_sqj")
    s2 = pool.tile([nparts, 1], F32, tag="ln_s2")
    nc.scalar.activation(sqj[:], pre, AF.Square, accum_out=s2[:])
    mu2 = pool.tile([nparts, 1], F32, tag="ln_mu2")
    ve.tensor_mul(mu2[:], mu[:], mu[:])
    var = pool.tile([nparts, 1], F32, tag="ln_var")
    ve.tensor_scalar(var[:], s2[:], 1.0 / dfeat, mu2[:], op0=ALU.mult, op1=ALU.subtract)
    # rstd = exp(-0.5*ln(var+eps)): keeps ACT in the natural_log_exp func set
    lnv = pool.tile([nparts, 1], F32, tag="ln_lnv")
    nc.scalar.activation(lnv[:], var[:], AF.Ln, bias=eps_ap)
    al = pool.tile([nparts, 1], F32, tag="ln_al")
    nc.scalar.activation(al[:], lnv[:], AF.Exp, scale=-0.5)
    mup = pool.tile([nparts, 1], F32, tag="ln_mup")
    ve.tensor_scalar(mup[:], mu[:], al[:], -1.0, op0=ALU.mult, op1=ALU.mult)
    xn = pool.tile([nparts, dfeat], F32, tag="ln_xn")
    ve.tensor_scalar(xn[:], pre, al[:], mup[:], op0=ALU.mult, op1=ALU.add)
    xg = pool.tile([nparts, dfeat], F32, tag="ln_xg")
    ve.tensor_mul(xg[:], xn[:], g_b)
    e2.tensor_add(out_ap, xg[:], b_b)
    return out_ap


def _build_body(nc, tc, D, out_d):
    import contextlib
    ctx = contextlib.ExitStack()
    ectx = contextlib.ExitStack()
    with ctx:
        cpool = ctx.enter_context(tc.tile_pool(name="const", bufs=1))
        w2pool = ectx.enter_context(tc.tile_pool(name="wts2", bufs=2))
        w1pool = ectx.enter_context(tc.tile_pool(name="wts1", bufs=1))
        apool = ectx.enter_context(tc.tile_pool(name="acts", bufs=1))
        spool = ectx.enter_context(tc.tile_pool(name="small", bufs=3))

        ident = cpool.tile([128, 128], F32, tag="ident")
        nc.sync.dma_start(out=ident[:], in_=D["identity"])
        mask = cpool.tile([128, NHEAD], F32, tag="mask")
        nc.sync.dma_start(out=mask[:], in_=D["mask"])
        maskT = cpool.tile([NHEAD, 128], F32, tag="maskT")
        nc.sync.dma_start(out=maskT[:], in_=D["maskT"])
        ones = cpool.tile([128, 1], F32, tag="ones_t")
        nc.sync.dma_start(out=ones[:], in_=D["ones"])
        eps_t = cpool.tile([128, 1], F32, tag="eps_t")
        nc.vector.memset(eps_t[:], EPS)

        # ---------------- encoder ----------------
        X_tok, XT = [], []
        for tt in range(2):
            xt_ = apool.tile([128, DIN], F32, tag=f"X_tok{tt}")
            nc.sync.dma_start(out=xt_[:], in_=D["src_tok"][tt * 128:(tt + 1) * 128, :])
            X_tok.append(xt_[:])
        for c in range(4):
            xc = apool.tile([128, NTOK], F32, tag=f"XT{c}")
            nc.sync.dma_start(out=xc[:], in_=D["srcT"][c * 128:(c + 1) * 128, :])
            XT.append(xc[:])

        for l in range(NENC):
            X_tok, XT = _enc_layer(nc, tc, D, l, X_tok, XT,
                                   w2pool, w1pool, apool, spool, ident, eps_t)

        # ---------------- fc + memory K/V ----------------
        fcTs = []
        for c in range(4):
            t_ = w1pool.tile([128, DOUT], F32, tag=f"fcT{c}")
            nc.sync.dma_start(out=t_[:], in_=D["fcT"][c * 128:(c + 1) * 128, :])
            fcTs.append(t_)
        bfc_b = cpool.tile([128, DOUT], F32, tag="bfc_b")
        _bcast_row(nc, cpool, D["bfc"], bfc_b, 128, "bfc")

        ckT = cpool.tile([DOUT, DOUT], F32, tag="d_ckT")
        nc.sync.dma_start(out=ckT[:], in_=D["d_ckT"])
        bck = cpool.tile([DOUT, 1], F32, tag="d_bck")
        nc.sync.dma_start(out=bck[:], in_=D["d_bck"])
        cvT = cpool.tile([DOUT, DOUT], F32, tag="d_cvT")
        nc.sync.dma_start(out=cvT[:], in_=D["d_cvT"])
        bcv_b = cpool.tile([128, DOUT], F32, tag="bcv_b")
        _bcast_row(nc, cpool, D["d_bcv"], bcv_b, 128, "bcv")

        Kmem = cpool.tile([128, NTOK], F32, tag="Kmem")
        Vmem = [cpool.tile([T, DOUT], F32, tag=f"Vmem{b}", name=f"Vmem{b}") for b in range(BL)]
        with tc.tile_pool(name="psfc", bufs=2, space="PSUM") as psfc:
            mem_tok = []
            for tt in range(2):
                mp = psfc.tile([128, DOUT], F32, tag="mem")
                for c in range(4):
                    nc.tensor.matmul(mp[:], XT[c][:, tt * 128:(tt + 1) * 128], fcTs[c][:],
                                     start=(c == 0), stop=(c == 3))
                ms = apool.tile([128, DOUT], F32, tag=f"mem_tok{tt}")
                nc.vector.tensor_add(ms[:], mp[:], bfc_b[:])
                mem_tok.append(ms)
            memT = apool.tile([128, NTOK], F32, tag="memT")
            for tt in range(2):
                tp = psfc.tile([128, 128], F32, tag="memTp")
                nc.tensor.transpose(tp[:], mem_tok[tt][:], ident[:])
                nc.scalar.copy(memT[:, tt * 128:(tt + 1) * 128], tp[:])
            kmp = psfc.tile([128, NTOK], F32, tag="kmem")
            nc.tensor.matmul(kmp[:], ckT[:], memT[:], start=True, stop=True)
            nc.scalar.activation(Kmem[:], kmp[:], AF.Identity, bias=bck[:])
            for b in range(BL):
                vmp = psfc.tile([T, DOUT], F32, tag="vmem")
                nc.tensor.matmul(vmp[:], memT[:, b * T:(b + 1) * T], cvT[:],
                                 start=True, stop=True)
                nc.vector.tensor_add(Vmem[b][:], vmp[:], bcv_b[0:T, :])

        # ---------------- decoder prep ----------------
        dw = {}
        for nm in ("d_sqT", "d_skT", "d_svT", "d_soT", "d_cqT", "d_coT"):
            t_ = cpool.tile([DOUT, DOUT], F32, tag=nm)
            nc.sync.dma_start(out=t_[:], in_=D[nm])
            dw[nm] = t_
        d_f1T = cpool.tile([DOUT, FF_DEC], F32, tag="d_f1T")
        nc.sync.dma_start(out=d_f1T[:], in_=D["d_f1T"])
        d_f2T = cpool.tile([FF_DEC, DOUT], F32, tag="d_f2T")
        nc.sync.dma_start(out=d_f2T[:], in_=D["d_f2T"])
        bsq = cpool.tile([DOUT, 1], F32, tag="d_bsq")
        nc.sync.dma_start(out=bsq[:], in_=D["d_bsq"])
        bsk = cpool.tile([DOUT, 1], F32, tag="d_bsk")
        nc.sync.dma_start(out=bsk[:], in_=D["d_bsk"])
        bcq = cpool.tile([DOUT, 1], F32, tag="d_bcq")
        nc.sync.dma_start(out=bcq[:], in_=D["d_bcq"])
        d_bf1 = cpool.tile([FF_DEC, 1], F32, tag="d_bf1")
        nc.sync.dma_start(out=d_bf1[:], in_=D["d_bf1"])
        gb = {}
        for nm in ("g1", "b1", "g2", "b2", "g3", "b3"):
            t_ = cpool.tile([DOUT, 1], F32, tag=f"c_{nm}")
            nc.sync.dma_start(out=t_[:], in_=D[f"d_{nm}c"])
            gb[nm] = t_
        rows = {}
        for nm in ("d_bso", "d_bco", "d_bf2"):
            r_ = cpool.tile([1, DOUT], F32, tag=f"row_{nm}")
            nc.sync.dma_start(out=r_[:], in_=D[nm])
            rows[nm] = r_
        ones_r = cpool.tile([1, 128], F32, tag="ones_r")
        nc.vector.memset(ones_r[:], 1.0)
        rows["ones_r"] = ones_r

        # K cache prefilled with k-bias (k of zero rows); V-delta cache zero.
        Kc = cpool.tile([128, BL * (T + 1)], F32, tag="Kc")
        nc.vector.tensor_copy(Kc[:], bsk[:].broadcast_to([128, BL * (T + 1)]))
        Vdc = cpool.tile([T + 1, BL * DOUT], F32, tag="Vdc")
        nc.vector.memset(Vdc[:], 0.0)

        ectx.close()   # release encoder-phase SBUF before the decode loop
        opool = ctx.enter_context(tc.tile_pool(name="outp", bufs=1))
        out_sb = opool.tile([BL, T * DOUT], F32, tag="out_sb")
        if NSTEP < T:
            nc.vector.memset(out_sb[:], 0.0)
        zeroT = cpool.tile([DOUT, BL], F32, tag="zeroT")
        nc.vector.memset(zeroT[:], 0.0)

        # ---------------- decode loop ----------------
        with tc.tile_pool(name="dstep", bufs=3) as dpool, \
             tc.tile_pool(name="psD", bufs=1, space="PSUM") as psD:
            xT = zeroT[:]
            vd_prev = None
            for t in range(NSTEP):
                xT, vd_prev = _dec_step(nc, t, xT, vd_prev, Kc, Vdc, Kmem, Vmem,
                                        dw, bsq, bsk, bcq, d_f1T, d_f2T, d_bf1,
                                        gb, rows, mask, maskT, ones, ident,
                                        dpool, psD, out_sb)

        nc.sync.dma_start(out=out_d.rearrange("b t d -> b (t d)"), in_=out_sb[:])


def _bcast_row(nc, cpool, dram_row, dst_tile, channels, key):
    row = cpool.tile([1, dram_row.shape[-1]], F32, tag=f"brow_{key}")
    nc.sync.dma_start(out=row[:], in_=dram_row)
    nc.gpsimd.partition_broadcast(dst_tile[:], row[:], channels=channels)


def _enc_layer(nc, tc, D, l, X_tok, XT, w2pool, w1pool, apool, spool, ident, eps_t):
    qkvT = []
    for c in range(4):
        t_ = w2pool.tile([128, 3 * DIN], F32, tag=f"qkvT{c}")
        nc.sync.dma_start(out=t_[:], in_=D[f"e{l}_qkvT"][c * 128:(c + 1) * 128, :])
        qkvT.append(t_)
    woT = []
    for c in range(4):
        t_ = w1pool.tile([128, DIN], F32, tag=f"woT{c}")
        nc.sync.dma_start(out=t_[:], in_=D[f"e{l}_woT"][c * 128:(c + 1) * 128, :])
        woT.append(t_)
    f1T = []
    for c in range(4):
        t_ = w1pool.tile([128, FF_ENC], F32, tag=f"f1T{c}")
        nc.sync.dma_start(out=t_[:], in_=D[f"e{l}_f1T"][c * 128:(c + 1) * 128, :])
        f1T.append(t_)
    f2T = w1pool.tile([FF_ENC, DIN], F32, tag="f2T")
    nc.sync.dma_start(out=f2T[:], in_=D[f"e{l}_f2T"])
    bqk = w1pool.tile([128, 8], F32, tag="bqk")
    nc.sync.dma_start(out=bqk[:], in_=D[f"e{l}_bqk"])
    bf1 = w1pool.tile([FF_ENC, 1], F32, tag="bf1")
    nc.sync.dma_start(out=bf1[:], in_=D[f"e{l}_bf1"])
    bb = {}
    for nm in ("bv", "bo", "bf2", "g1", "b1", "g2", "b2"):
        b_ = w1pool.tile([128, DIN], F32, tag=f"bb_{nm}")
        row = w1pool.tile([1, DIN], F32, tag=f"bbrow_{nm}")
        nc.sync.dma_start(out=row[:], in_=D[f"e{l}_{nm}"])
        nc.gpsimd.partition_broadcast(b_[:], row[:], channels=128)
        bb[nm] = b_

    # --- QKV ---
    QK = []
    V = []
    with tc.tile_pool(name=f"ps_qkv{l}", bufs=2, space="PSUM") as psq:
        for m in range(8):
            pq = psq.tile([128, NTOK], F32, tag="qk")
            for c in range(4):
                nc.tensor.matmul(pq[:], qkvT[c][:, m * 128:(m + 1) * 128], XT[c],
                                 start=(c == 0), stop=(c == 3))
            qs = apool.tile([128, NTOK], F32, tag=f"QK{m}")
            nc.scalar.activation(qs[:], pq[:], AF.Identity, bias=bqk[:, m:m + 1])
            QK.append(qs)
        for b in range(BL):
            pv = psq.tile([T, DIN], F32, tag="v")
            for c in range(4):
                nc.tensor.matmul(pv[:], XT[c][:, b * T:(b + 1) * T],
                                 qkvT[c][:, 2 * DIN:3 * DIN],
                                 start=(c == 0), stop=(c == 3))
            vs = apool.tile([T, DIN], F32, tag=f"V{b}", name=f"Vb{b}")
            nc.vector.tensor_add(vs[:], pv[:], bb["bv"][0:T, :])
            V.append(vs)

    # --- attention ---
    OT = [apool.tile([128, NTOK], F32, tag=f"OT{c}", name=f"OT{c}") for c in range(4)]
    with tc.tile_pool(name=f"ps_att{l}", bufs=2, space="PSUM") as psa, \
         tc.tile_pool(name=f"sb_att{l}", bufs=4) as sba:
        for b in range(BL):
            den = spool.tile([T, NHEAD], F32, tag="den")
            Ps = []
            for h in range(NHEAD):
                c, r0 = h // 2, (h % 2) * DHE
                Qs = QK[c][r0:r0 + DHE, b * T:(b + 1) * T]
                Ks = QK[4 + c][r0:r0 + DHE, b * T:(b + 1) * T]
                sp = psa.tile([T, T], F32, tag="S", bufs=3)
                nc.tensor.matmul(sp[:], Qs, Ks, start=True, stop=True)
                p_ = sba.tile([T, T], F32, tag="P", bufs=9)
                nc.scalar.activation(p_[:], sp[:], AF.Exp, scale=1.0 / np.sqrt(DHE),
                                     accum_out=den[:, h:h + 1])
                Ps.append(p_)
            rb = spool.tile([T, NHEAD], F32, tag="rb")
            nc.vector.reciprocal(rb[:], den[:])
            for h in range(NHEAD):
                c, r0 = h // 2, (h % 2) * DHE
                a_ = sba.tile([T, T], F32, tag="A")
                nc.scalar.activation(a_[:], Ps[h][:], AF.Copy, scale=rb[:, h:h + 1])
                atp = psa.tile([T, T], F32, tag="AT")
                nc.tensor.transpose(atp[:], a_[:], ident[0:T, 0:T])
                ats = sba.tile([T, T], F32, tag="ATs")
                nc.vector.tensor_copy(ats[:], atp[:])
                avp = psa.tile([DHE, T], F32, tag="AV")
                Vs = V[b][0:T, h * DHE:(h + 1) * DHE]
                nc.tensor.matmul(avp[:], Vs, ats[:], start=True, stop=True)
                nc.scalar.copy(OT[c][r0:r0 + DHE, b * T:(b + 1) * T], avp[:])

    # --- out-proj + residual + LN1 ---
    X1_tok = []
    X1T = [apool.tile([128, NTOK], F32, tag=f"X1T{c}", name=f"X1T{c}") for c in range(4)]
    with tc.tile_pool(name=f"ps_o{l}", bufs=2, space="PSUM") as pso:
        for tt in range(2):
            ap_ = pso.tile([128, DIN], F32, tag="ao")
            for c in range(4):
                nc.tensor.matmul(ap_[:], OT[c][:, tt * 128:(tt + 1) * 128], woT[c][:],
                                 start=(c == 0), stop=(c == 3))
            t1 = apool.tile([128, DIN], F32, tag=f"pre1_{tt}")
            nc.vector.tensor_add(t1[:], ap_[:], X_tok[tt])
            nc.vector.tensor_add(t1[:], t1[:], bb["bo"][:])
            x1 = apool.tile([128, DIN], F32, tag=f"X1_{tt}")
            _ln_tokmajor(nc, spool, t1[:], 128, DIN, bb["g1"][:], bb["b1"][:], x1[:], eps_t[:])
            X1_tok.append(x1[:])
        for tt in range(2):
            for c in range(4):
                tp = pso.tile([128, 128], F32, tag="xT")
                nc.tensor.transpose(tp[:], X1_tok[tt][:, c * 128:(c + 1) * 128], ident[:])
                nc.scalar.copy(X1T[c][:, tt * 128:(tt + 1) * 128], tp[:])

    # --- FFN + LN2 ---
    X2_tok = []
    X2T = [apool.tile([128, NTOK], F32, tag=f"X2T{c}", name=f"X2T{c}") for c in range(4)]
    with tc.tile_pool(name=f"ps_f{l}", bufs=2, space="PSUM") as psf:
        hp = psf.tile([FF_ENC, NTOK], F32, tag="h")
        for c in range(4):
            nc.tensor.matmul(hp[:], f1T[c][:], X1T[c][:], start=(c == 0), stop=(c == 3))
        hs = apool.tile([FF_ENC, NTOK], F32, tag="H")
        nc.scalar.activation(hs[:], hp[:], AF.Relu, bias=bf1[:])
        for tt in range(2):
            fp = psf.tile([128, DIN], F32, tag="f")
            nc.tensor.matmul(fp[:], hs[:, tt * 128:(tt + 1) * 128], f2T[:],
                             start=True, stop=True)
            t2 = apool.tile([128, DIN], F32, tag=f"pre2_{tt}")
            nc.vector.tensor_add(t2[:], fp[:], X1_tok[tt])
            nc.vector.tensor_add(t2[:], t2[:], bb["bf2"][:])
            x2 = apool.tile([128, DIN], F32, tag=f"X2_{tt}")
            _ln_tokmajor(nc, spool, t2[:], 128, DIN, bb["g2"][:], bb["b2"][:], x2[:], eps_t[:])
            X2_tok.append(x2[:])
        for tt in range(2):
            for c in range(4):
                tp = psf.tile([128, 128], F32, tag="xT2")
                nc.tensor.transpose(tp[:], X2_tok[tt][:, c * 128:(c + 1) * 128], ident[:])
                nc.scalar.copy(X2T[c][:, tt * 128:(tt + 1) * 128], tp[:])

    return X2_tok, X2T


def _attn_fm(nc, t, xT_ap, dpool, bank, ppb, qT, bq, K_ap, vlen, V_list, corr,
             mask, maskT, ones, brow_o, oT, ident, ones1b, pfx, scale):
    """Feature-major decoder attention sublayer.

    Returns the pre-LN residual PSUM tile AP [128, BL] (residual + bias +
    out-proj accumulated; caller copies to SBUF and layernorms).
    corr: list of (vd_rows: [4 x AP[1,128]], kcol: fn(b)->AP[128,1]) rank-1 V
    corrections; the exp'd score row is recomputed from the cache column.
    bank: a [128, 512] PSUM tile carved into column slices for this sublayer.
    ppb: a dedicated [128, BL] PSUM tile for the long-lived residual
    accumulation (CoreSim forbids reading a tensor with an open accum group).
    """
    qp = bank[0:DOUT, 0:BL]
    nc.tensor.matmul(qp, qT[:], xT_ap, start=True, stop=True)
    # residual accumulation starts early on its own PSUM tile
    pp = ppb[0:128, 0:BL]
    nc.tensor.matmul(pp, ident[:], xT_ap, start=True, stop=False)
    nc.tensor.matmul(pp, brow_o, ones1b, start=False, stop=False)
    # qblk = (q + bq) * head-mask, fused
    qblk = dpool.tile([128, BL * NHEAD], F32, tag=f"{pfx}qblk")
    nc.vector.scalar_tensor_tensor(
        out=qblk[:].rearrange("p (b h) -> p b h", b=BL),
        in0=qp.unsqueeze(2).broadcast_to([128, BL, NHEAD]),
        scalar=bq[:],
        in1=mask[:].unsqueeze(1).broadcast_to([128, BL, NHEAD]),
        op0=ALU.add, op1=ALU.mult)
    stp = bank[0:vlen, 8:8 + BL * NHEAD]
    for b in range(BL):
        nc.tensor.matmul(stp[:, b * NHEAD:(b + 1) * NHEAD],
                         K_ap[:, b * vlen:(b + 1) * vlen],
                         qblk[:, b * NHEAD:(b + 1) * NHEAD], start=True, stop=True)
    pt = dpool.tile([vlen, BL * NHEAD], F32, tag=f"{pfx}pt")
    nc.scalar.activation(pt[:], stp, AF.Exp, scale=scale)
    # exp'd score rows for the stale cache row (parallel to the main path)
    ptr = []
    for ci, (vdr, kcol) in enumerate(corr):
        srow = bank[0:1, 40 + 32 * ci:72 + 32 * ci]
        for b in range(BL):
            bsl = slice(b * NHEAD, (b + 1) * NHEAD)
            nc.tensor.matmul(srow[:, bsl], kcol(b), qblk[:, bsl],
                             start=True, stop=True)
        pr = dpool.tile([1, BL * NHEAD], F32, tag=f"{pfx}pr{ci}")
        nc.scalar.activation(pr[:], srow, AF.Exp, scale=scale)
        ptr.append(pr)
    denp = bank[0:NHEAD, 72:72 + BL]
    avp = bank[0:128, 76:76 + BL * NHEAD]
    for b in range(BL):
        bsl = slice(b * NHEAD, (b + 1) * NHEAD)
        nc.tensor.matmul(denp[:, b:b + 1], pt[:, bsl], ones[0:vlen, :],
                         start=True, stop=True)
        nc.tensor.matmul(avp[:, bsl], V_list[b], pt[:, bsl],
                         start=True, stop=(len(corr) == 0))
        for ci, (vdr, kcol) in enumerate(corr):
            nc.tensor.matmul(avp[:, bsl], vdr[b], ptr[ci][0:1, bsl],
                             start=False, stop=(ci == len(corr) - 1))
    r_ = dpool.tile([NHEAD, BL], F32, tag=f"{pfx}r")
    nc.vector.reciprocal(r_[:], denp)
    avm = dpool.tile([128, BL * NHEAD], F32, tag=f"{pfx}avm")
    nc.vector.tensor_mul(
        avm[:].rearrange("p (b h) -> p b h", b=BL),
        avp.rearrange("p (b h) -> p b h", b=BL),
        mask[:].unsqueeze(1).broadcast_to([128, BL, NHEAD]))
    o_ = dpool.tile([128, BL], F32, tag=f"{pfx}o")
    nc.vector.tensor_reduce(out=o_[:], in_=avm[:].rearrange("p (b h) -> p b h", b=BL),
                            op=ALU.add, axis=mybir.AxisListType.X)
    erp = bank[0:128, 108:108 + BL]
    nc.tensor.matmul(erp, maskT[:], r_[:], start=True, stop=True)
    on = dpool.tile([128, BL], F32, tag=f"{pfx}on")
    nc.vector.tensor_mul(on[:], o_[:], erp)
    nc.tensor.matmul(pp, oT[:], on[:], start=False, stop=True)
    return pp


def _ln_fm(nc, t, dpool, psD, pp, g_c, b_c, pfx):
    """Feature-major LN: copy PSUM->SBUF then fused gpsimd layernorm per col."""
    ps = dpool.tile([128, BL], F32, tag=f"{pfx}ps")
    nc.vector.tensor_copy(ps[:], pp[:])
    xo = dpool.tile([128, BL], F32, tag=f"{pfx}xo")
    for b in range(BL):
        nc.gpsimd.layernorm(out_ap=xo[:, b:b + 1], in_ap=ps[:, b:b + 1],
                            gamma_ap=g_c[:], beta_ap=b_c[:], eps=EPS,
                            subtract_mean=True, n_tokens=1)
    return xo


def _dec_step(nc, t, xT, vd_prev, Kc, Vdc, Kmem, Vmem, dw, bsq, bsk, bcq,
              d_f1T, d_f2T, d_bf1, gb, rows, mask, maskT, ones, ident,
              dpool, psD, out_sb):
    ones1b = rows["ones_r"][0:1, 0:BL]
    saB = psD.tile([128, 512], F32, tag="saB")
    caB = psD.tile([128, 512], F32, tag="caB")
    saP = psD.tile([128, BL], F32, tag="saP")
    caP = psD.tile([128, BL], F32, tag="caP")
    ffP = psD.tile([128, BL], F32, tag="ffP")
    mscB = psD.tile([128, 512], F32, tag="mscB")
    xtrB = psD.tile([BL, DOUT], F32, tag="xtrB")
    vrB = psD.tile([1, 512], F32, tag="vrB")

    # --- k/v projections into caches ---
    kp = mscB[0:DOUT, 0:BL]
    nc.tensor.matmul(kp, dw["d_skT"][:], xT, start=True, stop=True)
    kslice = Kc[:].rearrange("p (b j) -> p b j", b=BL)[:, :, t]
    nc.vector.tensor_scalar_add(kslice, kp, bsk[:])
    # v-delta rows (no bias), one [1,128] row per batch (matmul operands must
    # sit at partition 0), used for the rank-1 row-t correction + cache DMA
    for b in range(BL):
        nc.tensor.matmul(vrB[0:1, b * DOUT:(b + 1) * DOUT], xT[:, b:b + 1],
                         dw["d_svT"][:], start=True, stop=True)
    vd_sb = dpool.tile([1, BL * DOUT], F32, tag="vd_sb")
    nc.vector.tensor_copy(vd_sb[:], vrB[0:1, :])
    vd = [vd_sb[0:1, b * DOUT:(b + 1) * DOUT] for b in range(BL)]

    corr = [(vd, lambda b: Kc[:, b * (T + 1) + t:b * (T + 1) + t + 1])]
    Vd_list = [Vdc[:, b * DOUT:(b + 1) * DOUT] for b in range(BL)]
    pre1 = _attn_fm(nc, t, xT, dpool, saB, saP, dw["d_sqT"], bsq, Kc[:], T + 1,
                    Vd_list, corr, mask, maskT, ones, rows["d_bso"][:],
                    dw["d_soT"], ident, ones1b, "sa", 1.0 / np.sqrt(DHD))
    # cache row write for future steps (after this step's reads, off-path;
    # tile's RAW dep makes step t+1's AV wait for this DMA, which lands well
    # within one step's slack)
    for b in range(BL):
        nc.sync.dma_start(out=Vdc[t:t + 1, b * DOUT:(b + 1) * DOUT],
                          in_=vd[b])
    x1T = _ln_fm(nc, t, dpool, psD, pre1, gb["g1"], gb["b1"], "l1")

    Vm_list = [Vmem[b][:] for b in range(BL)]
    pre2 = _attn_fm(nc, t, x1T[:], dpool, caB, caP, dw["d_cqT"], bcq, Kmem[:], T,
                    Vm_list, [], mask, maskT, ones, rows["d_bco"][:],
                    dw["d_coT"], ident, ones1b, "ca", 1.0 / np.sqrt(DHD))
    x2T = _ln_fm(nc, t, dpool, psD, pre2, gb["g2"], gb["b2"], "l2")

    # --- FFN ---
    hp = mscB[0:FF_DEC, BL:2 * BL]
    nc.tensor.matmul(hp, d_f1T[:], x2T[:], start=True, stop=True)
    h_ = dpool.tile([FF_DEC, BL], F32, tag="hdec")
    nc.scalar.activation(h_[:], hp, AF.Relu, bias=d_bf1[:])
    pre3 = ffP[0:128, 0:BL]
    nc.tensor.matmul(pre3, ident[:], x2T[:], start=True, stop=False)
    nc.tensor.matmul(pre3, rows["d_bf2"][:], ones1b, start=False, stop=False)
    nc.tensor.matmul(pre3, d_f2T[:], h_[:], start=False, stop=True)
    xoT = _ln_fm(nc, t, dpool, psD, pre3, gb["g3"], gb["b3"], "l3")

    # --- emit output row (off critical path) ---
    xtr = xtrB[0:BL, 0:DOUT]
    nc.tensor.transpose(xtr, xoT[:], ident[:])
    nc.vector.tensor_copy(out_sb[:, t * DOUT:(t + 1) * DOUT], xtr)
    return xoT[:], None


# ------------------------------------------------------------------
# host side
# ------------------------------------------------------------------

def _prep_shared(inputs):
    f = np.ascontiguousarray
    S = {}
    for l in range(NLAYERS):
        qkv_w = inputs["enc_qkv_w"][l]
        S[f"e{l}_qkvT"] = f(qkv_w.T)
        qkv_b = inputs["enc_qkv_b"][l]
        S[f"e{l}_bqk"] = f(qkv_b[:2 * DIN].reshape(8, 128).T)
        S[f"e{l}_bv"] = f(qkv_b[2 * DIN:].reshape(1, DIN))
        S[f"e{l}_woT"] = f(inputs["enc_out_w"][l].T)
        S[f"e{l}_bo"] = f(inputs["enc_out_b"][l].reshape(1, DIN))
        S[f"e{l}_f1T"] = f(inputs["enc_ff1_w"][l].T)
        S[f"e{l}_bf1"] = f(inputs["enc_ff1_b"][l].reshape(FF_ENC, 1))
        S[f"e{l}_f2T"] = f(inputs["enc_ff2_w"][l].T)
        S[f"e{l}_bf2"] = f(inputs["enc_ff2_b"][l].reshape(1, DIN))
        S[f"e{l}_g1"] = f(inputs["enc_ln1_g"][l].reshape(1, DIN))
        S[f"e{l}_b1"] = f(inputs["enc_ln1_b"][l].reshape(1, DIN))
        S[f"e{l}_g2"] = f(inputs["enc_ln2_g"][l].reshape(1, DIN))
        S[f"e{l}_b2"] = f(inputs["enc_ln2_b"][l].reshape(1, DIN))
    S["fcT"] = f(inputs["fc_w"].T)
    S["bfc"] = f(inputs["fc_b"].reshape(1, DOUT))
    sq, sk, sv = np.split(inputs["dec_sa_qkv_w"], 3, axis=0)
    bq_, bk_, bv_ = np.split(inputs["dec_sa_qkv_b"], 3)
    S["d_sqT"] = f(sq.T); S["d_bsq"] = f(bq_.reshape(DOUT, 1))
    S["d_skT"] = f(sk.T); S["d_bsk"] = f(bk_.reshape(DOUT, 1))
    S["d_svT"] = f(sv.T)
    S["d_soT"] = f(inputs["dec_sa_out_w"].T)
    # softmax weights sum to 1, so the v-bias contributes exactly b_v to the
    # attention output; fold W_o @ b_v into the out-proj bias.
    S["d_bso"] = f((inputs["dec_sa_out_b"] + inputs["dec_sa_out_w"] @ bv_).reshape(1, DOUT))
    cq, ck, cv = np.split(inputs["dec_ca_qkv_w"], 3, axis=0)
    cbq, cbk, cbv = np.split(inputs["dec_ca_qkv_b"], 3)
    S["d_cqT"] = f(cq.T); S["d_bcq"] = f(cbq.reshape(DOUT, 1))
    S["d_ckT"] = f(ck.T); S["d_bck"] = f(cbk.reshape(DOUT, 1))
    S["d_cvT"] = f(cv.T); S["d_bcv"] = f(cbv.reshape(1, DOUT))
    S["d_coT"] = f(inputs["dec_ca_out_w"].T)
    S["d_bco"] = f(inputs["dec_ca_out_b"].reshape(1, DOUT))
    S["d_f1T"] = f(inputs["dec_ff1_w"].T)
    S["d_bf1"] = f(inputs["dec_ff1_b"].reshape(FF_DEC, 1))
    S["d_f2T"] = f(inputs["dec_ff2_w"].T)
    S["d_bf2"] = f(inputs["dec_ff2_b"].reshape(1, DOUT))
    for nm in ("g1", "b1", "g2", "b2", "g3", "b3"):
        S[f"d_{nm}c"] = f(inputs[f"dec_ln{nm[1]}_{nm[0]}"].reshape(DOUT, 1))
    S["identity"] = np.eye(128, dtype=np.float32)
    S["mask"] = (np.arange(128)[:, None] // DHD == np.arange(NHEAD)[None, :]).astype(np.float32)
    S["maskT"] = f(S["mask"].T)
    S["ones"] = np.ones((128, 1), dtype=np.float32)
    return {k: np.asarray(v, dtype=np.float32) for k, v in S.items()}


def make_in_maps(inputs):
    shared = _prep_shared(inputs)
    src = np.asarray(inputs["src"], dtype=np.float32)
    in_maps = []
    for c in range(NCORES):
        shard = np.ascontiguousarray(src[c * BL:(c + 1) * BL])
        tok = shard.reshape(NTOK, DIN)
        m = dict(shared)
        m["src_tok"] = np.ascontiguousarray(tok)
        m["srcT"] = np.ascontiguousarray(tok.T)
        in_maps.append(m)
    return in_maps


def kernel(**inputs) -> np.ndarray:
    from concourse.bass_utils import run_bass_kernel_spmd
    if "nc" not in _CACHE:
        _CACHE["nc"] = build_program()[0]
    nc = _CACHE["nc"]
    in_maps = make_in_maps(inputs)
    res = run_bass_kernel_spmd(nc, in_maps, core_ids=list(range(NCORES)))
    out = np.concatenate([r["out"] for r in res.results], axis=0)
    return out.astype(np.float32)
